# revision 37
# baseline (speedup 1.0000x reference)
"""Trainium2 Bass kernel for nn_Attention (dense transformer attention layer).

Sharding: 8 cores = 2 (batch) x 4 (head-group TP).  Core c handles batch
c//4 and heads [4*(c%4), 4*(c%4)+4).  Each core computes LayerNorm (in the
transposed domain, folded into augmented projection matmuls), q/k/v
projections, per-head RMS-norm'd attention, and a partial output
projection; the host sums the 4 partials per batch.

Precision: the dots have sigma~512 (scale=sqrt(dh) applied to BOTH q and k),
so softmax is near-argmax and the q/k path (projections + QK^T) must run in
fp32 on the PE (4 cyc/row).  The v path, attention weights, and Wo run in
bf16.

Layout notes:
 - x is fed transposed (x^T [DIM, S]) so all matmuls contract over
   partitions without on-device transposition of x.
 - LayerNorm: mean/var per token are computed with ones-stationary matmuls
   (bf16 copy of x^T), then folded into the projections via two augmented
   contraction rows: z = [x^T; colsums; 1/r], W = [ln_w*W; -u/1024; ln_b@W].
   The per-token 1/r factor cancels in q/k (RMSNorm scale invariance) and is
   applied explicitly to v.
 - RMSNorm of q is folded into the softmax exp (ACT scale/bias are
   per-partition APs); RMSNorm of k is applied via a broadcast multiply.
"""
import numpy as np
import ml_dtypes
import os
from contextlib import ExitStack

import concourse.bass as bass
import concourse.tile as tile
from concourse import mybir
from concourse.bass_utils import run_bass_kernel_spmd
from concourse.masks import make_identity

F32 = mybir.dt.float32
F32R = mybir.dt.float32r
BF16 = mybir.dt.bfloat16
_OF = F32R if os.environ.get("OUT_F32R", "0") == "1" else F32
AF = mybir.ActivationFunctionType
ALU = mybir.AluOpType
AX = mybir.AxisListType

B, S, DIM, H, DH = 2, 2048, 1024, 16, 64
NCORES = 8
HPC = 4                  # heads per core
INC = HPC * DH           # 256 inner dims per core
KCH = DIM // 128         # 8 contraction chunks of x
NT = 2                   # q/k/v^T tiles per core ([128, S] each, 2 heads per tile)
SCH = S // 128           # 16 token chunks
NJ = S // 512            # 4 moving chunks

_TPB_ENGINES = None


def _fix_multiwaits(nc, max_waits=1):
    """walrus in this container encodes at most one semaphore wait per TPB
    instruction; split extras onto single-wait NoOps ahead of the
    instruction (same engine => program order preserves semantics)."""
    global _TPB_ENGINES
    if _TPB_ENGINES is None:
        _TPB_ENGINES = {
            mybir.EngineType.PE,
            mybir.EngineType.Activation,
            mybir.EngineType.DVE,
            mybir.EngineType.Pool,
            mybir.EngineType.SP,
        }
    nsplit = 0
    for f in nc.m.functions:
        for bb in f.blocks:
            new = []
            for inst in bb.instructions:
                si = inst.sync_info
                if (
                    inst.engine in _TPB_ENGINES
                    and si is not None
                    and si.on_wait
                    and len(si.on_wait) > max_waits
                ):
                    waits = list(si.on_wait)
                    extra, keep = waits[:-max_waits], waits[-max_waits:]
                    for w in extra:
                        nop = mybir.InstNoOp(
                            name=f"I-{nc.next_id()}",
                            ins=[],
                            outs=[],
                            engine=inst.engine,
                            sync_info=mybir.SyncInfo(on_wait=[w], on_update=[]),
                        )
                        try:
                            nc.register_instruction(nop, overwrite=True)
                        except Exception:
                            pass
                        new.append(nop)
                    try:
                        si.on_wait[:] = keep
                    except TypeError:
                        inst.sync_info = mybir.SyncInfo(
                            on_wait=keep, on_update=si.on_update
                        )
                    nsplit += 1
                new.append(inst)
            bb.instructions[:] = new
    return nsplit


DEBUG_DUMPS = False


def _build_program():
    nc = bass.Bass("TRN2", target_bir_lowering=False, debug=False,
                   num_devices=NCORES)
    din = lambda n, s, d: nc.dram_tensor(n, list(s), d, kind="ExternalInput").ap()
    xT = din("xT", (DIM, S), F32)
    xTb = din("xTb", (DIM, S), BF16)
    wq_d = din("Wq", (9 * 128, INC), F32)
    wk_d = din("Wk", (9 * 128, INC), F32)
    wv_d = din("Wv", (9 * 128, INC), BF16)
    wo_d = din("Wo", (INC, DIM), _OF)
    gq_d = din("gq", (INC, 1), F32)
    gk_d = din("gk", (INC, 1), F32)
    e2_d = din("E2", (128, 2), F32)
    e2t_d = din("E2T", (2, 128), F32)
    out_d = nc.dram_tensor("out", [S, DIM], F32, kind="ExternalOutput").ap()
    packs_dram = nc.dram_tensor("packs_scratch", [12 * 128, S], BF16,
                                kind="Internal").ap()
    if DEBUG_DUMPS:
        dbg_q = nc.dram_tensor("dbg_q", [128, S], F32, kind="ExternalOutput").ap()
        dbg_k = nc.dram_tensor("dbg_k", [128, S], F32, kind="ExternalOutput").ap()
        dbg_rq = nc.dram_tensor("dbg_rq", [128, 2 * SCH], F32,
                                kind="ExternalOutput").ap()
        dbg_v = nc.dram_tensor("dbg_v", [128, INC], BF16,
                               kind="ExternalOutput").ap()
        dbg_rc = nc.dram_tensor("dbg_rc", [128, SCH], F32,
                                kind="ExternalOutput").ap()
        dbg_at = nc.dram_tensor("dbg_at", [128, S], BF16,
                                kind="ExternalOutput").ap()
        dbg_ot = nc.dram_tensor("dbg_ot", [128, S], BF16,
                                kind="ExternalOutput").ap()

    with tile.TileContext(nc) as tc, ExitStack() as ctx:
        # ---- long-lived pools
        consts = ctx.enter_context(tc.tile_pool(name="consts", bufs=1))
        vpool = ctx.enter_context(tc.tile_pool(name="v", bufs=1))

        ident = consts.tile([128, 128], BF16, tag="ident", name="ident")
        make_identity(nc, ident)
        ident_f = consts.tile([128, 128], F32, tag="ident_f", name="ident_f")
        make_identity(nc, ident_f)
        ones_b = consts.tile([128, 1], BF16, tag="ones", name="ones")
        nc.vector.memset(ones_b, 1.0)
        e2 = consts.tile([128, 2], F32, tag="e2", name="e2")
        nc.sync.dma_start(out=e2, in_=e2_d[:])
        e2t2 = consts.tile([2, 128], F32, tag="e2t2", name="e2t2")
        nc.sync.dma_start(out=e2t2, in_=e2t_d[:])
        r_col = consts.tile([128, SCH], F32, tag="r_col", name="r_col")
        gq_t = [consts.tile([128, 1], F32, tag=f"gq{t}", name=f"gq{t}")
                for t in range(NT)]
        gk_t = [consts.tile([128, 1], F32, tag=f"gk{t}", name=f"gk{t}")
                for t in range(NT)]
        for t in range(NT):
            nc.sync.dma_start(out=gq_t[t], in_=gq_d[t * 128:(t + 1) * 128, :])
            nc.sync.dma_start(out=gk_t[t], in_=gk_d[t * 128:(t + 1) * 128, :])
        eps5 = consts.tile([128, 1], F32, tag="eps5", name="eps5")
        nc.vector.memset(eps5, 1e-5)
        aug_f = consts.tile([2, S], F32, tag="aug_f", name="aug_f")
        nc.vector.memset(aug_f, 0.0)
        r_sb = consts.tile([1, S], F32, tag="r_sb", name="r_sb")

        # rmsnorm factors for q (per-partition layout; col = 2*sc+hh)
        rq_all = [consts.tile([128, 2 * SCH], F32, tag=f"rq{t}", name=f"rq{t}")
                  for t in range(NT)]
        nrq_all = [consts.tile([128, 2 * SCH], F32, tag=f"nrq{t}", name=f"nrq{t}")
                   for t in range(NT)]
        rs_all = [consts.tile([128, 2 * SCH], F32, tag=f"rs{t}", name=f"rs{t}")
                  for t in range(NT)]

        v_nat = [vpool.tile([128, INC], BF16, tag=f"vn{j}", name=f"vn{j}")
                 for j in range(SCH)]


        with ExitStack() as phase_bcde:
            qkpool = phase_bcde.enter_context(tc.tile_pool(name="qk", bufs=1))
            qhat = [qkpool.tile([128, S], F32, tag=f"qh{t}", name=f"qh{t}")
                    for t in range(NT)]
            khat = [qkpool.tile([128, S], F32, tag=f"kh{t}", name=f"kh{t}")
                    for t in range(NT)]

            with ExitStack() as phase_bc:
                xpool = phase_bc.enter_context(tc.tile_pool(name="x", bufs=1))
                wpool = phase_bc.enter_context(tc.tile_pool(name="w", bufs=1))
                xt = [xpool.tile([128, S], F32, tag=f"xt{k}", name=f"xt{k}")
                      for k in range(KCH)]
                wq = [wpool.tile([128, INC], F32, tag=f"wq{k}", name=f"wq{k}")
                      for k in range(9)]
                wk = [wpool.tile([128, INC], F32, tag=f"wk{k}", name=f"wk{k}")
                      for k in range(9)]

                # ---- phase B: LayerNorm stats (bf16 x^T streamed) ---------
                with ExitStack() as phase_b:
                    xbpool = phase_b.enter_context(
                        tc.tile_pool(name="xb", bufs=2))
                    x2pool = phase_b.enter_context(
                        tc.tile_pool(name="x2", bufs=1))
                    browp = phase_b.enter_context(
                        tc.tile_pool(name="brow", bufs=1))
                    stps = phase_b.enter_context(
                        tc.tile_pool(name="stps", bufs=1, space="PSUM"))
                    for k in range(KCH):
                        nc.sync.dma_start(out=xt[k],
                                          in_=xT[k * 128:(k + 1) * 128, :])
                    sums_ps = stps.tile([1, S], F32, tag="sums", name="sums")
                    sumsq_ps = stps.tile([1, S], F32, tag="sumsq", name="sumsq")
                    for k in range(KCH):
                        xb = xbpool.tile([128, S], BF16, tag="xb", name="xb")
                        nc.sync.dma_start(out=xb,
                                          in_=xTb[k * 128:(k + 1) * 128, :])
                        # x_lo = x - bf16(x): second bf16x2 term so the token
                        # means are fp32-accurate (mu noise shifts dots)
                        xl = x2pool.tile([128, S], BF16, tag="xl", name="xl")
                        nc.vector.tensor_sub(xl, xt[k], xb)
                        x2 = x2pool.tile([128, S], BF16, tag="x2", name="x2")
                        nc.scalar.square(x2, xb)
                        for n in range(NJ):
                            nsl = slice(n * 512, (n + 1) * 512)
                            nc.tensor.matmul(sums_ps[:, nsl], ones_b,
                                             xb[:, nsl],
                                             start=(k == 0), stop=False,
                                             skip_group_check=True)
                            nc.tensor.matmul(sums_ps[:, nsl], ones_b,
                                             xl[:, nsl],
                                             start=False,
                                             stop=(k == KCH - 1),
                                             skip_group_check=True)
                            nc.tensor.matmul(sumsq_ps[:, nsl], ones_b,
                                             x2[:, nsl],
                                             start=(k == 0),
                                             stop=(k == KCH - 1),
                                             skip_group_check=True)
                    for k in range(9):
                        nc.sync.dma_start(out=wq[k],
                                          in_=wq_d[k * 128:(k + 1) * 128, :])
                        nc.sync.dma_start(out=wk[k],
                                          in_=wk_d[k * 128:(k + 1) * 128, :])
                    # b = sumsq - sums^2/1024  (all [1, S])
                    tmp = browp.tile([1, S], F32, tag="strow", name="strow")
                    nc.vector.tensor_scalar_mul(tmp, sums_ps, 1.0 / DIM)
                    nc.vector.tensor_mul(tmp, tmp, sums_ps)
                    nc.vector.tensor_sub(tmp, sumsq_ps, tmp)
                    # t = b/1024 + 1e-5 (= var+eps); r = rsqrt(t) via Newton
                    tv = browp.tile([1, S], F32, tag="tv", name="tv")
                    nc.vector.tensor_scalar(tv, tmp, 1.0 / DIM, 1e-5,
                                            op0=ALU.mult, op1=ALU.add)
                    nc.scalar.activation(out=tmp, in_=tv, func=AF.Sqrt,
                                         bias=0.0, scale=1.0)
                    nc.vector.reciprocal(r_sb, tmp)
                    nc.vector.tensor_mul(tmp, r_sb, r_sb)
                    nc.vector.tensor_mul(tmp, tmp, tv)
                    nc.vector.tensor_scalar(tmp, tmp, -0.5, 1.5,
                                            op0=ALU.mult, op1=ALU.add)
                    nc.vector.tensor_mul(r_sb, r_sb, tmp)
                    # invr = t * r
                    nc.vector.tensor_mul(tv, tv, r_sb)
                    nc.vector.tensor_copy(aug_f[0:1, :], sums_ps)
                    nc.sync.dma_start(out=aug_f[1:2, :], in_=tv)

                # ---- phase C: q/k projections (fp32) ----------------------
                with ExitStack() as phase_c:
                    prps = phase_c.enter_context(
                        tc.tile_pool(name="prps", bufs=4, space="PSUM"))
                    for wt, dst in ((wq, qhat), (wk, khat)):
                        for m in range(NT):
                            msl = slice(m * 128, (m + 1) * 128)
                            for n in range(NJ):
                                nsl = slice(n * 512, (n + 1) * 512)
                                ps = prps.tile([128, 512], F32, tag="proj",
                                               name="proj")
                                for k in range(KCH):
                                    nc.tensor.matmul(ps, wt[k][:, msl],
                                                     xt[k][:, nsl],
                                                     start=(k == 0),
                                                     stop=False)
                                nc.tensor.matmul(ps, wt[8][0:2, msl],
                                                 aug_f[:, nsl],
                                                 start=False, stop=True)
                                if n % 2 == 0:
                                    nc.vector.tensor_copy(dst[m][:, nsl], ps)
                                else:
                                    nc.scalar.copy(dst[m][:, nsl], ps)

            # ---- phase D: rmsnorm factors + bf16x2 packs -----------------
            with ExitStack() as phase_d:
                sq2pool = phase_d.enter_context(
                    tc.tile_pool(name="sq2", bufs=2))
                dstage = phase_d.enter_context(
                    tc.tile_pool(name="dstage", bufs=1))
                ssqps = phase_d.enter_context(
                    tc.tile_pool(name="ssqps", bufs=2, space="PSUM"))
                sskps = phase_d.enter_context(
                    tc.tile_pool(name="sskps", bufs=1, space="PSUM"))
                kmps = phase_d.enter_context(
                    tc.tile_pool(name="kmps", bufs=2, space="PSUM"))
                for t in range(NT):
                    q2 = sq2pool.tile([128, S], F32, tag="q2", name="q2")
                    nc.gpsimd.tensor_mul(q2, qhat[t], qhat[t])
                    for sc in range(SCH):
                        psq = ssqps.tile([128, 2], F32, tag="ssq", name="ssq")
                        nc.tensor.matmul(psq, q2[:, sc * 128:(sc + 1) * 128],
                                         e2, start=True, stop=True)
                        nc.vector.tensor_copy(
                            rq_all[t][:, 2 * sc:2 * sc + 2], psq)
                    # rq = rsqrt(ss/64 + 1e-8), Newton-refined
                    tq = dstage.tile([128, 2 * SCH], F32, tag="tq", name="tq")
                    nc.vector.tensor_scalar(tq, rq_all[t], 1.0 / DH, 1e-8,
                                            op0=ALU.mult, op1=ALU.add)
                    nc.scalar.activation(out=rq_all[t], in_=tq, func=AF.Sqrt,
                                         bias=0.0, scale=1.0)
                    nc.vector.reciprocal(rq_all[t], rq_all[t])
                    zz = dstage.tile([128, 2 * SCH], F32, tag="zz", name="zz")
                    nc.vector.tensor_mul(zz, rq_all[t], rq_all[t])
                    nc.vector.tensor_mul(zz, zz, tq)
                    nc.vector.tensor_scalar(zz, zz, -0.5, 1.5,
                                            op0=ALU.mult, op1=ALU.add)
                    nc.vector.tensor_mul(rq_all[t], rq_all[t], zz)
                    nc.vector.tensor_scalar_mul(nrq_all[t], rq_all[t], -1.0)
                    # gamma*8 on q
                    nc.vector.tensor_scalar_mul(qhat[t], qhat[t], gq_t[t])

                    k2t = sq2pool.tile([128, S], F32, tag="q2", name="k2t")
                    nc.gpsimd.tensor_mul(k2t, khat[t], khat[t])
                    psk = sskps.tile([2, S], F32, tag="ssk", name="ssk")
                    for n in range(NJ):
                        nsl = slice(n * 512, (n + 1) * 512)
                        nc.tensor.matmul(psk[:, nsl], e2, k2t[:, nsl],
                                         start=True, stop=True,
                                         skip_group_check=True)
                    rk2 = dstage.tile([2, S], F32, tag="rk2", name="rk2")
                    tk = dstage.tile([2, S], F32, tag="tk", name="tk")
                    nc.vector.tensor_scalar(tk, psk, 1.0 / DH, 1e-8,
                                            op0=ALU.mult, op1=ALU.add)
                    nc.scalar.activation(out=rk2, in_=tk, func=AF.Sqrt,
                                         bias=0.0, scale=1.0)
                    nc.vector.reciprocal(rk2, rk2)
                    zk = dstage.tile([2, S], F32, tag="zk", name="zk")
                    nc.vector.tensor_mul(zk, rk2, rk2)
                    nc.vector.tensor_mul(zk, zk, tk)
                    nc.vector.tensor_scalar(zk, zk, -0.5, 1.5,
                                            op0=ALU.mult, op1=ALU.add)
                    nc.vector.tensor_mul(rk2, rk2, zk)
                    # gamma*8 on k, then rk broadcast multiply (K=2 matmul)
                    nc.vector.tensor_scalar_mul(khat[t], khat[t], gk_t[t])
                    for n in range(NJ):
                        nsl = slice(n * 512, (n + 1) * 512)
                        km = kmps.tile([128, 512], F32, tag="km", name="km")
                        nc.tensor.matmul(km, e2t2, rk2[:, nsl],
                                         start=True, stop=True)
                        nc.vector.tensor_mul(khat[t][:, nsl],
                                             khat[t][:, nsl], km)

                # bf16x2 packs (hi/lo split at native base) -> DRAM scratch
                # rows: h*128 block; q_pack h -> block h, k1 -> 4+h, k2 -> 8+h
                lostage = phase_d.enter_context(
                    tc.tile_pool(name="lost", bufs=2))
                for t in range(NT):
                    for hh in range(2):
                        h4 = 2 * t + hh
                        rows = slice(hh * 64, hh * 64 + 64)
                        qb = slice(h4 * 128, h4 * 128 + 64)
                        qb2 = slice(h4 * 128 + 64, h4 * 128 + 128)
                        k1b = slice((4 + h4) * 128, (4 + h4) * 128 + 64)
                        k1b2 = slice((4 + h4) * 128 + 64, (4 + h4) * 128 + 128)
                        k2b = slice((8 + h4) * 128, (8 + h4) * 128 + 64)
                        k2b2 = slice((8 + h4) * 128 + 64, (8 + h4) * 128 + 128)
                        hi = lostage.tile([128, S], BF16, tag="hi", name="hi")
                        lo = lostage.tile([128, S], BF16, tag="lo", name="lo")
                        nc.vector.tensor_copy(hi[rows, :], qhat[t][rows, :])
                        nc.vector.tensor_sub(lo[rows, :], qhat[t][rows, :],
                                             hi[rows, :])
                        nc.sync.dma_start(out=packs_dram[qb, :],
                                          in_=hi[rows, :])
                        nc.sync.dma_start(out=packs_dram[qb2, :],
                                          in_=lo[rows, :])
                        hi2 = lostage.tile([128, S], BF16, tag="hi",
                                           name="hi2")
                        lo2 = lostage.tile([128, S], BF16, tag="lo",
                                           name="lo2")
                        nc.vector.tensor_copy(hi2[rows, :], khat[t][rows, :])
                        nc.vector.tensor_sub(lo2[rows, :], khat[t][rows, :],
                                             hi2[rows, :])
                        nc.sync.dma_start(out=packs_dram[k1b, :],
                                          in_=hi2[rows, :])
                        nc.sync.dma_start(out=packs_dram[k1b2, :],
                                          in_=hi2[rows, :])
                        nc.sync.dma_start(out=packs_dram[k2b, :],
                                          in_=lo2[rows, :])
                        nc.sync.dma_start(out=packs_dram[k2b2, :],
                                          in_=lo2[rows, :])

                if DEBUG_DUMPS:
                    nc.sync.dma_start(out=dbg_q[:], in_=qhat[0])
                    nc.sync.dma_start(out=dbg_k[:], in_=khat[0])
                    nc.sync.dma_start(out=dbg_rq[:], in_=rq_all[0])

            # ---- phase E: v projection (bf16, k-outer) + v transposes ----
            with ExitStack() as phase_e:
                xbpool2 = phase_e.enter_context(tc.tile_pool(name="xb2",
                                                             bufs=2))
                wvpool = phase_e.enter_context(tc.tile_pool(name="wv", bufs=1))
                vtpool = phase_e.enter_context(tc.tile_pool(name="vT", bufs=1))
                phase_e1 = phase_e.enter_context(ExitStack())
                vprps = phase_e1.enter_context(
                    tc.tile_pool(name="vprps", bufs=1, space="PSUM"))
                aug_b = wvpool.tile([2, S], BF16, tag="aug_b", name="aug_b")
                nc.gpsimd.dma_start(out=aug_b, in_=aug_f)
                wv = [wvpool.tile([128, INC], BF16, tag=f"wv{k}",
                                  name=f"wv{k}") for k in range(9)]
                for k in range(9):
                    nc.sync.dma_start(out=wv[k],
                                      in_=wv_d[k * 128:(k + 1) * 128, :])
                vT = [vtpool.tile([128, S], F32, tag=f"vT{t}", name=f"vT{t}")
                      for t in range(NT)]
                vps = [vprps.tile([128, 512], F32, tag=f"vp{i}", name=f"vp{i}")
                       for i in range(NT * NJ)]
                for k in range(KCH):
                    xb = xbpool2.tile([128, S], BF16, tag="xb2", name="xb2")
                    nc.sync.dma_start(out=xb,
                                      in_=xTb[k * 128:(k + 1) * 128, :])
                    for m in range(NT):
                        msl = slice(m * 128, (m + 1) * 128)
                        for n in range(NJ):
                            nsl = slice(n * 512, (n + 1) * 512)
                            nc.tensor.matmul(vps[m * NJ + n], wv[k][:, msl],
                                             xb[:, nsl],
                                             start=(k == 0), stop=False)
                for m in range(NT):
                    msl = slice(m * 128, (m + 1) * 128)
                    for n in range(NJ):
                        nsl = slice(n * 512, (n + 1) * 512)
                        nc.tensor.matmul(vps[m * NJ + n], wv[8][0:2, msl],
                                         aug_b[:, nsl], start=False, stop=True)
                        if n % 2 == 0:
                            nc.vector.tensor_copy(vT[m][:, nsl],
                                                  vps[m * NJ + n])
                        else:
                            nc.scalar.copy(vT[m][:, nsl], vps[m * NJ + n])

                # r_col + v transposes; fold r into the psum->v_nat copy so
                # v is rounded to bf16 exactly once
                phase_e1.close()
                with ExitStack() as phase_e2:
                    vtps = phase_e2.enter_context(
                        tc.tile_pool(name="vtps", bufs=2, space="PSUM"))
                    rtp = phase_e2.enter_context(
                        tc.tile_pool(name="rtp", bufs=2, space="PSUM"))
                    for j in range(SCH):
                        rp = rtp.tile([128, 1], F32, tag="rp", name="rp")
                        nc.tensor.transpose(rp,
                                            r_sb[0:1, j * 128:(j + 1) * 128],
                                            ident_f[0:1, 0:1])
                        nc.vector.tensor_copy(r_col[:, j:j + 1], rp)
                    for t in range(NT):
                        for j in range(SCH):
                            tp = vtps.tile([128, 128], F32, tag="vtp",
                                           name="vtp")
                            nc.tensor.transpose(
                                tp, vT[t][:, j * 128:(j + 1) * 128], ident_f)
                            nc.vector.tensor_scalar_mul(
                                v_nat[j][:, t * 128:(t + 1) * 128], tp,
                                r_col[:, j:j + 1])

        if DEBUG_DUMPS:
            nc.sync.dma_start(out=dbg_v[:], in_=v_nat[0])
            nc.sync.dma_start(out=dbg_rc[:], in_=r_col)

        opool = ctx.enter_context(tc.tile_pool(name="o", bufs=1))
        outT = [opool.tile([128, S], _OF, tag=f"oT{k}", name=f"oT{k}")
                for k in range(2)]
        packp = ctx.enter_context(tc.tile_pool(name="packs", bufs=1))
        q_pack = [packp.tile([128, S], BF16, tag=f"qp{h}", name=f"qp{h}")
                  for h in range(HPC)]
        k1_pack = [packp.tile([128, S], BF16, tag=f"k1p{h}", name=f"k1p{h}")
                   for h in range(HPC)]
        k2_pack = [packp.tile([128, S], BF16, tag=f"k2p{h}", name=f"k2p{h}")
                   for h in range(HPC)]
        for h in range(HPC):
            nc.sync.dma_start(out=q_pack[h],
                              in_=packs_dram[h * 128:(h + 1) * 128, :])
            nc.sync.dma_start(out=k1_pack[h],
                              in_=packs_dram[(4 + h) * 128:(5 + h) * 128, :])
            nc.sync.dma_start(out=k2_pack[h],
                              in_=packs_dram[(8 + h) * 128:(9 + h) * 128, :])

        # ---- phase F: attention ------------------------------------------
        with ExitStack() as phase_f:
            dots_pool = phase_f.enter_context(
                tc.tile_pool(name="dots", bufs=2, space="PSUM"))
            tpps = phase_f.enter_context(
                tc.tile_pool(name="tpps", bufs=2, space="PSUM"))
            avps = phase_f.enter_context(
                tc.tile_pool(name="avps", bufs=1, space="PSUM"))
            attn_pool = phase_f.enter_context(tc.tile_pool(name="attn",
                                                           bufs=3))
            attnT_pool = phase_f.enter_context(tc.tile_pool(name="attnT",
                                                            bufs=2))
            small = phase_f.enter_context(tc.tile_pool(name="small", bufs=8))

            for t in range(NT):
                for hh in range(2):
                    h4 = 2 * t + hh
                    for sup in range(SCH // 4):
                        attnT = attnT_pool.tile([128, 4 * S], BF16,
                                                tag="attnT", name="attnT")
                        for u in range(4):
                            ic = sup * 4 + u
                            isl = slice(ic * 128, (ic + 1) * 128)
                            col = slice(2 * ic + hh, 2 * ic + hh + 1)
                            dots = [dots_pool.tile([128, 1024], F32,
                                                   tag="dots", name="dots")
                                    for _ in range(2)]
                            for jn in range(NJ):
                                d = dots[jn // 2]
                                dsl = slice((jn % 2) * 512,
                                            (jn % 2) * 512 + 512)
                                jsl = slice(jn * 512, (jn + 1) * 512)
                                nc.tensor.matmul(
                                    d[:, dsl], q_pack[h4][:, isl],
                                    k1_pack[h4][:, jsl],
                                    start=True, stop=False,
                                    skip_group_check=True)
                                nc.tensor.matmul(
                                    d[:, dsl], q_pack[h4][:, isl],
                                    k2_pack[h4][:, jsl],
                                    start=False, stop=True,
                                    skip_group_check=True)
                            mx = [small.tile([128, 1], F32, tag=f"mx{j}",
                                             name=f"mx{j}")
                                  for j in range(2)]
                            for jn in range(2):
                                nc.vector.tensor_reduce(out=mx[jn],
                                                        in_=dots[jn],
                                                        axis=AX.X, op=ALU.max)
                            nc.vector.tensor_max(mx[0], mx[0], mx[1])
                            bias = small.tile([128, 1], F32, tag="bias",
                                              name="bias")
                            nc.vector.tensor_mul(bias, mx[0],
                                                 nrq_all[t][:, col])
                            attn = attn_pool.tile([128, S], BF16, tag="attn",
                                                  name="attn")
                            sm = [small.tile([128, 1], F32, tag=f"sm{j}",
                                             name=f"sm{j}")
                                  for j in range(2)]
                            for jn in range(2):
                                jsl = slice(jn * 1024, (jn + 1) * 1024)
                                nc.scalar.activation(
                                    out=attn[:, jsl], in_=dots[jn],
                                    func=AF.Exp, bias=bias,
                                    scale=rq_all[t][:, col],
                                    accum_out=sm[jn])
                            nc.vector.tensor_add(sm[0], sm[0], sm[1])
                            # store 1/sum; normalization deferred to out^T
                            nc.vector.reciprocal(rs_all[t][:, col], sm[0])
                            if DEBUG_DUMPS and t == 0 and hh == 0 and \
                                    sup == 0 and u == 0:
                                nc.sync.dma_start(out=dbg_at[:], in_=attn)
                            for jq in range(SCH // 4):
                                tp = tpps.tile([128, 512], BF16, tag="tp",
                                               name="tp")
                                for j2 in range(4):
                                    jc = jq * 4 + j2
                                    nc.tensor.transpose(
                                        tp[:, j2 * 128:(j2 + 1) * 128],
                                        attn[:, jc * 128:(jc + 1) * 128],
                                        ident)
                                for j2 in range(4):
                                    jc = jq * 4 + j2
                                    dsl2 = slice(jc * 512 + u * 128,
                                                 jc * 512 + u * 128 + 128)
                                    nc.vector.tensor_copy(
                                        attnT[:, dsl2],
                                        tp[:, j2 * 128:(j2 + 1) * 128])
                        av = avps.tile([64, 512], F32, tag="av", name="av")
                        for jc in range(SCH):
                            nc.tensor.matmul(
                                av, v_nat[jc][:, h4 * 64:(h4 + 1) * 64],
                                attnT[:, jc * 512:(jc + 1) * 512],
                                start=(jc == 0), stop=(jc == SCH - 1))
                        poff = hh * 64
                        ssl = slice(sup * 512, (sup + 1) * 512)
                        if sup % 2 == 0:
                            nc.vector.tensor_copy(outT[t][poff:poff + 64, ssl],
                                                  av)
                        else:
                            nc.scalar.copy(outT[t][poff:poff + 64, ssl], av)

        # ---- phase F2: normalize out^T by 1/sum ---------------------------
        with ExitStack() as phase_f2:
            rowps = phase_f2.enter_context(
                tc.tile_pool(name="rowps", bufs=2, space="PSUM"))
            bcps = phase_f2.enter_context(
                tc.tile_pool(name="bcps", bufs=2, space="PSUM"))
            rrow = phase_f2.enter_context(tc.tile_pool(name="rrow", bufs=2))
            for t in range(NT):
                rowA = rrow.tile([1, S], F32, tag="rowA", name="rowA")
                rowB = rrow.tile([1, S], F32, tag="rowB", name="rowB")
                for hh, row in ((0, rowA), (1, rowB)):
                    for nq in range(NJ):
                        rp = rowps.tile([1, 512], F32, tag="rp", name="rp")
                        for sc4 in range(4):
                            sc = nq * 4 + sc4
                            col = slice(2 * sc + hh, 2 * sc + hh + 1)
                            nc.tensor.transpose(
                                rp[:, sc4 * 128:(sc4 + 1) * 128],
                                rs_all[t][:, col], ident_f)
                        nc.vector.tensor_copy(
                            row[:, nq * 512:(nq + 1) * 512], rp)
                rs2 = rrow.tile([2, S], F32, tag="rs2", name="rs2")
                nc.sync.dma_start(out=rs2[0:1, :], in_=rowA)
                nc.sync.dma_start(out=rs2[1:2, :], in_=rowB)
                for nq in range(NJ):
                    nsl = slice(nq * 512, (nq + 1) * 512)
                    bc = bcps.tile([128, 512], F32, tag="bc", name="bc")
                    nc.tensor.matmul(bc, e2t2, rs2[:, nsl],
                                     start=True, stop=True)
                    nc.vector.tensor_mul(outT[t][:, nsl], outT[t][:, nsl], bc)

        if DEBUG_DUMPS:
            nc.sync.dma_start(out=dbg_ot[:], in_=outT[0])

        # ---- phase G: output projection (f32r) ---------------------------
        with ExitStack() as phase_g:
            wops = phase_g.enter_context(
                tc.tile_pool(name="wops", bufs=4, space="PSUM"))
            gpool = phase_g.enter_context(tc.tile_pool(name="g", bufs=1))
            ostage = phase_g.enter_context(tc.tile_pool(name="ost", bufs=2))

            wo = [gpool.tile([128, DIM], _OF, tag=f"wo{k}", name=f"wo{k}")
                  for k in range(2)]
            for k in range(2):
                nc.sync.dma_start(out=wo[k], in_=wo_d[k * 128:(k + 1) * 128, :])
            for sc in range(SCH):
                ssl = slice(sc * 128, (sc + 1) * 128)
                ost = ostage.tile([128, DIM], F32, tag="ost", name="ost")
                for nn in range(2):
                    nsl = slice(nn * 512, (nn + 1) * 512)
                    ps = wops.tile([128, 512], F32, tag="wops", name="wops")
                    for kk in range(2):
                        nc.tensor.matmul(ps, outT[kk][:, ssl], wo[kk][:, nsl],
                                         start=(kk == 0), stop=(kk == 1))
                    if nn % 2 == 0:
                        nc.vector.tensor_copy(ost[:, nsl], ps)
                    else:
                        nc.scalar.copy(ost[:, nsl], ps)
                nc.sync.dma_start(out=out_d[ssl, :], in_=ost)

    _fix_multiwaits(nc)
    return nc


_NC = None


def _get_nc():
    global _NC
    if _NC is None:
        _NC = _build_program()
    return _NC


def kernel(x, ln_w, ln_b, Wq, Wkv, q_gamma, k_gamma, Wo):
    x = np.asarray(x, np.float32)
    ln_w = np.asarray(ln_w, np.float32)
    ln_b = np.asarray(ln_b, np.float32)
    Wq = np.asarray(Wq, np.float32)
    Wkv = np.asarray(Wkv, np.float32)
    q_gamma = np.asarray(q_gamma, np.float32)
    k_gamma = np.asarray(k_gamma, np.float32)
    Wo = np.asarray(Wo, np.float32)
    Wk_full = Wkv[:, :H * DH]
    Wv_full = Wkv[:, H * DH:]

    bf = ml_dtypes.bfloat16
    e2_host = np.zeros((128, 2), np.float32)
    e2_host[0:64, 0] = 1.0
    e2_host[64:128, 1] = 1.0
    e2t_host = np.ascontiguousarray(e2_host.T)

    def aug_weights(Wsl):
        # [1152, INC]: [ln_w*W; -colsum/1024; ln_b@W; zeros]
        Wt = ln_w[:, None] * Wsl
        out = np.zeros((9 * 128, INC), np.float32)
        out[:DIM] = Wt
        out[DIM] = -Wt.sum(axis=0) / DIM
        out[DIM + 1] = ln_b @ Wsl
        return out

    in_maps = []
    for c in range(NCORES):
        b = c // (NCORES // B)
        g0 = (c % (NCORES // B)) * HPC
        hsl = slice(g0 * DH, (g0 + HPC) * DH)
        xt_host = np.ascontiguousarray(x[b].T)
        in_maps.append({
            "xT": xt_host,
            "xTb": xt_host.astype(bf),
            "Wq": aug_weights(Wq[:, hsl]),
            "Wk": aug_weights(Wk_full[:, hsl]),
            "Wv": aug_weights(Wv_full[:, hsl]).astype(bf),
            "Wo": np.ascontiguousarray(Wo[hsl, :]) if _OF != BF16
                  else np.ascontiguousarray(Wo[hsl, :]).astype(bf),
            "gq": (8.0 * q_gamma[g0:g0 + HPC]).reshape(INC, 1).astype(np.float32),
            "gk": (8.0 * k_gamma[g0:g0 + HPC]).reshape(INC, 1).astype(np.float32),
            "E2": e2_host,
            "E2T": e2t_host,
        })

    res = run_bass_kernel_spmd(_get_nc(), in_maps, list(range(NCORES))).results
    gpb = NCORES // B
    out = np.zeros((B, S, DIM), np.float32)
    for b in range(B):
        acc = np.zeros((S, DIM), np.float32)
        for c in range(b * gpb, (b + 1) * gpb):
            acc += res[c]["out"]
        out[b] = acc
    return out


# revision 41
# speedup vs baseline: 1.1478x; 1.1478x over previous
"""Trainium2 Bass kernel for nn_Attention (dense transformer attention layer).

Sharding: 8 cores = 2 (batch) x 4 (head-group TP).  Core c handles batch
c//4 and heads [4*(c%4), 4*(c%4)+4).  Each core computes LayerNorm (in the
transposed domain, folded into augmented projection matmuls), q/k/v
projections, per-head RMS-norm'd attention, and a partial output
projection; the host sums the 4 partials per batch.

Precision: the dots have sigma~512 (scale=sqrt(dh) applied to BOTH q and k),
so softmax is near-argmax and the q/k path (projections + QK^T) must run in
fp32 on the PE (4 cyc/row).  The v path, attention weights, and Wo run in
bf16.

Layout notes:
 - x is fed transposed (x^T [DIM, S]) so all matmuls contract over
   partitions without on-device transposition of x.
 - LayerNorm: mean/var per token are computed with ones-stationary matmuls
   (bf16 copy of x^T), then folded into the projections via two augmented
   contraction rows: z = [x^T; colsums; 1/r], W = [ln_w*W; -u/1024; ln_b@W].
   The per-token 1/r factor cancels in q/k (RMSNorm scale invariance) and is
   applied explicitly to v.
 - RMSNorm of q is folded into the softmax exp (ACT scale/bias are
   per-partition APs); RMSNorm of k is applied via a broadcast multiply.
"""
import numpy as np
import ml_dtypes
import os
from contextlib import ExitStack

import concourse.bass as bass
import concourse.tile as tile
from concourse import mybir
from concourse.bass_utils import run_bass_kernel_spmd
from concourse.masks import make_identity

F32 = mybir.dt.float32
F32R = mybir.dt.float32r
BF16 = mybir.dt.bfloat16
_OF = F32R if os.environ.get("OUT_F32R", "0") == "1" else F32
AF = mybir.ActivationFunctionType
ALU = mybir.AluOpType
AX = mybir.AxisListType

B, S, DIM, H, DH = 2, 2048, 1024, 16, 64
NCORES = 8
HPC = 4                  # heads per core
INC = HPC * DH           # 256 inner dims per core
KCH = DIM // 128         # 8 contraction chunks of x
NT = 2                   # q/k/v^T tiles per core ([128, S] each, 2 heads per tile)
SCH = S // 128           # 16 token chunks
NJ = S // 512            # 4 moving chunks

_TPB_ENGINES = None


def _fix_multiwaits(nc, max_waits=1):
    """walrus in this container encodes at most one semaphore wait per TPB
    instruction; split extras onto single-wait NoOps ahead of the
    instruction (same engine => program order preserves semantics)."""
    global _TPB_ENGINES
    if _TPB_ENGINES is None:
        _TPB_ENGINES = {
            mybir.EngineType.PE,
            mybir.EngineType.Activation,
            mybir.EngineType.DVE,
            mybir.EngineType.Pool,
            mybir.EngineType.SP,
        }
    nsplit = 0
    for f in nc.m.functions:
        for bb in f.blocks:
            new = []
            for inst in bb.instructions:
                si = inst.sync_info
                if (
                    inst.engine in _TPB_ENGINES
                    and si is not None
                    and si.on_wait
                    and len(si.on_wait) > max_waits
                ):
                    waits = list(si.on_wait)
                    extra, keep = waits[:-max_waits], waits[-max_waits:]
                    for w in extra:
                        nop = mybir.InstNoOp(
                            name=f"I-{nc.next_id()}",
                            ins=[],
                            outs=[],
                            engine=inst.engine,
                            sync_info=mybir.SyncInfo(on_wait=[w], on_update=[]),
                        )
                        try:
                            nc.register_instruction(nop, overwrite=True)
                        except Exception:
                            pass
                        new.append(nop)
                    try:
                        si.on_wait[:] = keep
                    except TypeError:
                        inst.sync_info = mybir.SyncInfo(
                            on_wait=keep, on_update=si.on_update
                        )
                    nsplit += 1
                new.append(inst)
            bb.instructions[:] = new
    return nsplit


DEBUG_DUMPS = False


def _build_program():
    nc = bass.Bass("TRN2", target_bir_lowering=False, debug=False,
                   num_devices=NCORES)
    din = lambda n, s, d: nc.dram_tensor(n, list(s), d, kind="ExternalInput").ap()
    xT = din("xT", (DIM, S), F32)
    xTb = din("xTb", (DIM, S), BF16)
    wq_d = din("Wq", (9 * 128, INC), F32)
    wk_d = din("Wk", (9 * 128, INC), F32)
    wv_d = din("Wv", (9 * 128, INC), BF16)
    wo_d = din("Wo", (INC, DIM), _OF)
    gq_d = din("gq", (INC, 1), F32)
    gk_d = din("gk", (INC, 1), F32)
    e2_d = din("E2", (128, 2), F32)
    e2t_d = din("E2T", (2, 128), F32)
    out_d = nc.dram_tensor("out", [S, DIM], F32, kind="ExternalOutput").ap()
    packs_dram = nc.dram_tensor("packs_scratch", [12 * 128, S], BF16,
                                kind="Internal").ap()
    if DEBUG_DUMPS:
        dbg_q = nc.dram_tensor("dbg_q", [128, S], F32, kind="ExternalOutput").ap()
        dbg_k = nc.dram_tensor("dbg_k", [128, S], F32, kind="ExternalOutput").ap()
        dbg_rq = nc.dram_tensor("dbg_rq", [128, 2 * SCH], F32,
                                kind="ExternalOutput").ap()
        dbg_v = nc.dram_tensor("dbg_v", [128, INC], BF16,
                               kind="ExternalOutput").ap()
        dbg_rc = nc.dram_tensor("dbg_rc", [128, SCH], F32,
                                kind="ExternalOutput").ap()
        dbg_at = nc.dram_tensor("dbg_at", [128, S], BF16,
                                kind="ExternalOutput").ap()
        dbg_ot = nc.dram_tensor("dbg_ot", [128, S], BF16,
                                kind="ExternalOutput").ap()

    with tile.TileContext(nc) as tc, ExitStack() as ctx:
        # ---- long-lived pools
        consts = ctx.enter_context(tc.tile_pool(name="consts", bufs=1))
        vpool = ctx.enter_context(tc.tile_pool(name="v", bufs=1))

        ident = consts.tile([128, 128], BF16, tag="ident", name="ident")
        make_identity(nc, ident)
        ident_f = consts.tile([128, 128], F32, tag="ident_f", name="ident_f")
        make_identity(nc, ident_f)
        ones_b = consts.tile([128, 1], BF16, tag="ones", name="ones")
        nc.vector.memset(ones_b, 1.0)
        e2 = consts.tile([128, 2], F32, tag="e2", name="e2")
        nc.sync.dma_start(out=e2, in_=e2_d[:])
        e2t2 = consts.tile([2, 128], F32, tag="e2t2", name="e2t2")
        nc.sync.dma_start(out=e2t2, in_=e2t_d[:])
        r_col = consts.tile([128, SCH], F32, tag="r_col", name="r_col")
        gq_t = [consts.tile([128, 1], F32, tag=f"gq{t}", name=f"gq{t}")
                for t in range(NT)]
        gk_t = [consts.tile([128, 1], F32, tag=f"gk{t}", name=f"gk{t}")
                for t in range(NT)]
        for t in range(NT):
            nc.sync.dma_start(out=gq_t[t], in_=gq_d[t * 128:(t + 1) * 128, :])
            nc.sync.dma_start(out=gk_t[t], in_=gk_d[t * 128:(t + 1) * 128, :])
        eps5 = consts.tile([128, 1], F32, tag="eps5", name="eps5")
        nc.vector.memset(eps5, 1e-5)
        aug_f = consts.tile([2, S], F32, tag="aug_f", name="aug_f")
        nc.vector.memset(aug_f, 0.0)
        r_sb = consts.tile([1, S], F32, tag="r_sb", name="r_sb")

        # rmsnorm factors for q (per-partition layout; col = 2*sc+hh)
        rq_all = [consts.tile([128, 2 * SCH], F32, tag=f"rq{t}", name=f"rq{t}")
                  for t in range(NT)]
        nrq_all = [consts.tile([128, 2 * SCH], F32, tag=f"nrq{t}", name=f"nrq{t}")
                   for t in range(NT)]
        rs_all = [consts.tile([128, 2 * SCH], F32, tag=f"rs{t}", name=f"rs{t}")
                  for t in range(NT)]

        v_nat = [vpool.tile([128, INC], BF16, tag=f"vn{j}", name=f"vn{j}")
                 for j in range(SCH)]


        with ExitStack() as phase_bcde:
            qkpool = phase_bcde.enter_context(tc.tile_pool(name="qk", bufs=1))
            qhat = [qkpool.tile([128, S], F32, tag=f"qh{t}", name=f"qh{t}")
                    for t in range(NT)]
            khat = [qkpool.tile([128, S], F32, tag=f"kh{t}", name=f"kh{t}")
                    for t in range(NT)]

            with ExitStack() as phase_bc:
                xpool = phase_bc.enter_context(tc.tile_pool(name="x", bufs=1))
                wpool = phase_bc.enter_context(tc.tile_pool(name="w", bufs=1))
                xt = [xpool.tile([128, S], F32, tag=f"xt{k}", name=f"xt{k}")
                      for k in range(KCH)]
                wq = [wpool.tile([128, INC], F32, tag=f"wq{k}", name=f"wq{k}")
                      for k in range(9)]
                wk = [wpool.tile([128, INC], F32, tag=f"wk{k}", name=f"wk{k}")
                      for k in range(9)]

                # ---- phase B: LayerNorm stats (bf16 x^T streamed) ---------
                with ExitStack() as phase_b:
                    xbpool = phase_b.enter_context(
                        tc.tile_pool(name="xb", bufs=2))
                    x2pool = phase_b.enter_context(
                        tc.tile_pool(name="x2", bufs=1))
                    browp = phase_b.enter_context(
                        tc.tile_pool(name="brow", bufs=1))
                    stps = phase_b.enter_context(
                        tc.tile_pool(name="stps", bufs=1, space="PSUM"))
                    sums_ps = stps.tile([1, S], F32, tag="sums", name="sums")
                    sumsq_ps = stps.tile([1, S], F32, tag="sumsq", name="sumsq")
                    for k in range(KCH):
                        nc.sync.dma_start(out=xt[k],
                                          in_=xT[k * 128:(k + 1) * 128, :])
                        xb = xbpool.tile([128, S], BF16, tag="xb", name="xb")
                        nc.sync.dma_start(out=xb,
                                          in_=xTb[k * 128:(k + 1) * 128, :])
                        # x_lo = x - bf16(x): second bf16x2 term so the token
                        # means are fp32-accurate (mu noise shifts dots)
                        xl = x2pool.tile([128, S], BF16, tag="xl", name="xl")
                        nc.vector.tensor_sub(xl, xt[k], xb)
                        x2 = x2pool.tile([128, S], BF16, tag="x2", name="x2")
                        nc.scalar.square(x2, xb)
                        for n in range(NJ):
                            nsl = slice(n * 512, (n + 1) * 512)
                            nc.tensor.matmul(sums_ps[:, nsl], ones_b,
                                             xb[:, nsl],
                                             start=(k == 0), stop=False,
                                             skip_group_check=True)
                            nc.tensor.matmul(sums_ps[:, nsl], ones_b,
                                             xl[:, nsl],
                                             start=False,
                                             stop=(k == KCH - 1),
                                             skip_group_check=True)
                            nc.tensor.matmul(sumsq_ps[:, nsl], ones_b,
                                             x2[:, nsl],
                                             start=(k == 0),
                                             stop=(k == KCH - 1),
                                             skip_group_check=True)
                    for k in range(9):
                        nc.sync.dma_start(out=wq[k],
                                          in_=wq_d[k * 128:(k + 1) * 128, :])
                        nc.sync.dma_start(out=wk[k],
                                          in_=wk_d[k * 128:(k + 1) * 128, :])
                    # b = sumsq - sums^2/1024  (all [1, S])
                    tmp = browp.tile([1, S], F32, tag="strow", name="strow")
                    nc.vector.tensor_scalar_mul(tmp, sums_ps, 1.0 / DIM)
                    nc.vector.tensor_mul(tmp, tmp, sums_ps)
                    nc.vector.tensor_sub(tmp, sumsq_ps, tmp)
                    # t = b/1024 + 1e-5 (= var+eps); r = rsqrt(t) via Newton
                    tv = browp.tile([1, S], F32, tag="tv", name="tv")
                    nc.vector.tensor_scalar(tv, tmp, 1.0 / DIM, 1e-5,
                                            op0=ALU.mult, op1=ALU.add)
                    nc.scalar.activation(out=tmp, in_=tv, func=AF.Sqrt,
                                         bias=0.0, scale=1.0)
                    nc.vector.reciprocal(r_sb, tmp)
                    nc.vector.tensor_mul(tmp, r_sb, r_sb)
                    nc.vector.tensor_mul(tmp, tmp, tv)
                    nc.vector.tensor_scalar(tmp, tmp, -0.5, 1.5,
                                            op0=ALU.mult, op1=ALU.add)
                    nc.vector.tensor_mul(r_sb, r_sb, tmp)
                    # invr = t * r
                    nc.vector.tensor_mul(tv, tv, r_sb)
                    nc.vector.tensor_copy(aug_f[0:1, :], sums_ps)
                    nc.sync.dma_start(out=aug_f[1:2, :], in_=tv)

                # ---- phase C: q/k projections (fp32) ----------------------
                with ExitStack() as phase_c:
                    prps = phase_c.enter_context(
                        tc.tile_pool(name="prps", bufs=4, space="PSUM"))
                    for wt, dst in ((wq, qhat), (wk, khat)):
                        for m in range(NT):
                            msl = slice(m * 128, (m + 1) * 128)
                            for n in range(NJ):
                                nsl = slice(n * 512, (n + 1) * 512)
                                ps = prps.tile([128, 512], F32, tag="proj",
                                               name="proj")
                                for k in range(KCH):
                                    nc.tensor.matmul(ps, wt[k][:, msl],
                                                     xt[k][:, nsl],
                                                     start=(k == 0),
                                                     stop=False)
                                nc.tensor.matmul(ps, wt[8][0:2, msl],
                                                 aug_f[:, nsl],
                                                 start=False, stop=True)
                                if n % 2 == 0:
                                    nc.vector.tensor_copy(dst[m][:, nsl], ps)
                                else:
                                    nc.scalar.copy(dst[m][:, nsl], ps)

            # ---- phase E: v projection (bf16, k-outer) + v transposes ----
            with ExitStack() as phase_e:
                xbpool2 = phase_e.enter_context(tc.tile_pool(name="xb2",
                                                             bufs=2))
                wvpool = phase_e.enter_context(tc.tile_pool(name="wv", bufs=1))
                vtpool = phase_e.enter_context(tc.tile_pool(name="vT", bufs=1))
                phase_e1 = phase_e.enter_context(ExitStack())
                vprps = phase_e1.enter_context(
                    tc.tile_pool(name="vprps", bufs=1, space="PSUM"))
                aug_b = wvpool.tile([2, S], BF16, tag="aug_b", name="aug_b")
                nc.gpsimd.dma_start(out=aug_b, in_=aug_f)
                wv = [wvpool.tile([128, INC], BF16, tag=f"wv{k}",
                                  name=f"wv{k}") for k in range(9)]
                for k in range(9):
                    nc.sync.dma_start(out=wv[k],
                                      in_=wv_d[k * 128:(k + 1) * 128, :])
                vT = [vtpool.tile([128, S], F32, tag=f"vT{t}", name=f"vT{t}")
                      for t in range(NT)]
                vps = [vprps.tile([128, 512], F32, tag=f"vp{i}", name=f"vp{i}")
                       for i in range(NT * NJ)]
                for k in range(KCH):
                    xb = xbpool2.tile([128, S], BF16, tag="xb2", name="xb2")
                    nc.sync.dma_start(out=xb,
                                      in_=xTb[k * 128:(k + 1) * 128, :])
                    for m in range(NT):
                        msl = slice(m * 128, (m + 1) * 128)
                        for n in range(NJ):
                            nsl = slice(n * 512, (n + 1) * 512)
                            nc.tensor.matmul(vps[m * NJ + n], wv[k][:, msl],
                                             xb[:, nsl],
                                             start=(k == 0), stop=False)
                for m in range(NT):
                    msl = slice(m * 128, (m + 1) * 128)
                    for n in range(NJ):
                        nsl = slice(n * 512, (n + 1) * 512)
                        nc.tensor.matmul(vps[m * NJ + n], wv[8][0:2, msl],
                                         aug_b[:, nsl], start=False, stop=True)
                        if n % 2 == 0:
                            nc.vector.tensor_copy(vT[m][:, nsl],
                                                  vps[m * NJ + n])
                        else:
                            nc.scalar.copy(vT[m][:, nsl], vps[m * NJ + n])

                # r_col + v transposes; fold r into the psum->v_nat copy so
                # v is rounded to bf16 exactly once
                phase_e1.close()
                with ExitStack() as phase_e2:
                    vtps = phase_e2.enter_context(
                        tc.tile_pool(name="vtps", bufs=2, space="PSUM"))
                    rtp = phase_e2.enter_context(
                        tc.tile_pool(name="rtp", bufs=2, space="PSUM"))
                    for j in range(SCH):
                        rp = rtp.tile([128, 1], F32, tag="rp", name="rp")
                        nc.tensor.transpose(rp,
                                            r_sb[0:1, j * 128:(j + 1) * 128],
                                            ident_f[0:1, 0:1])
                        nc.vector.tensor_copy(r_col[:, j:j + 1], rp)
                    for t in range(NT):
                        for j in range(SCH):
                            tp = vtps.tile([128, 128], F32, tag="vtp",
                                           name="vtp")
                            nc.tensor.transpose(
                                tp, vT[t][:, j * 128:(j + 1) * 128], ident_f)
                            nc.vector.tensor_scalar_mul(
                                v_nat[j][:, t * 128:(t + 1) * 128], tp,
                                r_col[:, j:j + 1])

            # ---- phase D: rmsnorm factors + bf16x2 packs -----------------
            with ExitStack() as phase_d:
                sq2pool = phase_d.enter_context(
                    tc.tile_pool(name="sq2", bufs=2))
                dstage = phase_d.enter_context(
                    tc.tile_pool(name="dstage", bufs=1))
                ssqps = phase_d.enter_context(
                    tc.tile_pool(name="ssqps", bufs=2, space="PSUM"))
                sskps = phase_d.enter_context(
                    tc.tile_pool(name="sskps", bufs=1, space="PSUM"))
                kmps = phase_d.enter_context(
                    tc.tile_pool(name="kmps", bufs=2, space="PSUM"))
                for t in range(NT):
                    q2 = sq2pool.tile([128, S], F32, tag="q2", name="q2")
                    nc.gpsimd.tensor_mul(q2, qhat[t], qhat[t])
                    for sc in range(SCH):
                        psq = ssqps.tile([128, 2], F32, tag="ssq", name="ssq")
                        nc.tensor.matmul(psq, q2[:, sc * 128:(sc + 1) * 128],
                                         e2, start=True, stop=True)
                        nc.vector.tensor_copy(
                            rq_all[t][:, 2 * sc:2 * sc + 2], psq)
                    # rq = rsqrt(ss/64 + 1e-8), Newton-refined
                    tq = dstage.tile([128, 2 * SCH], F32, tag="tq", name="tq")
                    nc.vector.tensor_scalar(tq, rq_all[t], 1.0 / DH, 1e-8,
                                            op0=ALU.mult, op1=ALU.add)
                    nc.scalar.activation(out=rq_all[t], in_=tq, func=AF.Sqrt,
                                         bias=0.0, scale=1.0)
                    nc.vector.reciprocal(rq_all[t], rq_all[t])
                    zz = dstage.tile([128, 2 * SCH], F32, tag="zz", name="zz")
                    nc.vector.tensor_mul(zz, rq_all[t], rq_all[t])
                    nc.vector.tensor_mul(zz, zz, tq)
                    nc.vector.tensor_scalar(zz, zz, -0.5, 1.5,
                                            op0=ALU.mult, op1=ALU.add)
                    nc.vector.tensor_mul(rq_all[t], rq_all[t], zz)
                    nc.vector.tensor_scalar_mul(nrq_all[t], rq_all[t], -1.0)
                    # gamma*8 on q
                    nc.vector.tensor_scalar_mul(qhat[t], qhat[t], gq_t[t])

                    k2t = sq2pool.tile([128, S], F32, tag="q2", name="k2t")
                    nc.gpsimd.tensor_mul(k2t, khat[t], khat[t])
                    psk = sskps.tile([2, S], F32, tag="ssk", name="ssk")
                    for n in range(NJ):
                        nsl = slice(n * 512, (n + 1) * 512)
                        nc.tensor.matmul(psk[:, nsl], e2, k2t[:, nsl],
                                         start=True, stop=True,
                                         skip_group_check=True)
                    rk2 = dstage.tile([2, S], F32, tag="rk2", name="rk2")
                    tk = dstage.tile([2, S], F32, tag="tk", name="tk")
                    nc.vector.tensor_scalar(tk, psk, 1.0 / DH, 1e-8,
                                            op0=ALU.mult, op1=ALU.add)
                    nc.scalar.activation(out=rk2, in_=tk, func=AF.Sqrt,
                                         bias=0.0, scale=1.0)
                    nc.vector.reciprocal(rk2, rk2)
                    zk = dstage.tile([2, S], F32, tag="zk", name="zk")
                    nc.vector.tensor_mul(zk, rk2, rk2)
                    nc.vector.tensor_mul(zk, zk, tk)
                    nc.vector.tensor_scalar(zk, zk, -0.5, 1.5,
                                            op0=ALU.mult, op1=ALU.add)
                    nc.vector.tensor_mul(rk2, rk2, zk)
                    # gamma*8 on k, then rk broadcast multiply (K=2 matmul)
                    nc.vector.tensor_scalar_mul(khat[t], khat[t], gk_t[t])
                    for n in range(NJ):
                        nsl = slice(n * 512, (n + 1) * 512)
                        km = kmps.tile([128, 512], F32, tag="km", name="km")
                        nc.tensor.matmul(km, e2t2, rk2[:, nsl],
                                         start=True, stop=True)
                        nc.vector.tensor_mul(khat[t][:, nsl],
                                             khat[t][:, nsl], km)

                # bf16x2 packs (hi/lo split at native base) -> DRAM scratch
                lostage = phase_d.enter_context(
                    tc.tile_pool(name="lost", bufs=2))
                for t in range(NT):
                    for hh in range(2):
                        h4 = 2 * t + hh
                        rows = slice(hh * 64, hh * 64 + 64)
                        qb = slice(h4 * 128, h4 * 128 + 64)
                        qb2 = slice(h4 * 128 + 64, h4 * 128 + 128)
                        k1b = slice((4 + h4) * 128, (4 + h4) * 128 + 64)
                        k1b2 = slice((4 + h4) * 128 + 64, (4 + h4) * 128 + 128)
                        k2b = slice((8 + h4) * 128, (8 + h4) * 128 + 64)
                        k2b2 = slice((8 + h4) * 128 + 64, (8 + h4) * 128 + 128)
                        hi = lostage.tile([128, S], BF16, tag="hi", name="hi")
                        lo = lostage.tile([128, S], BF16, tag="lo", name="lo")
                        nc.vector.tensor_copy(hi[rows, :], qhat[t][rows, :])
                        nc.vector.tensor_sub(lo[rows, :], qhat[t][rows, :],
                                             hi[rows, :])
                        nc.sync.dma_start(out=packs_dram[qb, :],
                                          in_=hi[rows, :])
                        nc.sync.dma_start(out=packs_dram[qb2, :],
                                          in_=lo[rows, :])
                        hi2 = lostage.tile([128, S], BF16, tag="hi",
                                           name="hi2")
                        lo2 = lostage.tile([128, S], BF16, tag="lo",
                                           name="lo2")
                        nc.vector.tensor_copy(hi2[rows, :], khat[t][rows, :])
                        nc.vector.tensor_sub(lo2[rows, :], khat[t][rows, :],
                                             hi2[rows, :])
                        nc.sync.dma_start(out=packs_dram[k1b, :],
                                          in_=hi2[rows, :])
                        nc.sync.dma_start(out=packs_dram[k1b2, :],
                                          in_=hi2[rows, :])
                        nc.sync.dma_start(out=packs_dram[k2b, :],
                                          in_=lo2[rows, :])
                        nc.sync.dma_start(out=packs_dram[k2b2, :],
                                          in_=lo2[rows, :])

                if DEBUG_DUMPS:
                    nc.sync.dma_start(out=dbg_q[:], in_=qhat[0])
                    nc.sync.dma_start(out=dbg_k[:], in_=khat[0])
                    nc.sync.dma_start(out=dbg_rq[:], in_=rq_all[0])

        if DEBUG_DUMPS:
            nc.sync.dma_start(out=dbg_v[:], in_=v_nat[0])
            nc.sync.dma_start(out=dbg_rc[:], in_=r_col)

        opool = ctx.enter_context(tc.tile_pool(name="o", bufs=1))
        outT = [opool.tile([128, S], _OF, tag=f"oT{k}", name=f"oT{k}")
                for k in range(2)]
        packp = ctx.enter_context(tc.tile_pool(name="packs", bufs=1))
        q_pack = [packp.tile([128, S], BF16, tag=f"qp{h}", name=f"qp{h}")
                  for h in range(HPC)]
        k1_pack = [packp.tile([128, S], BF16, tag=f"k1p{h}", name=f"k1p{h}")
                   for h in range(HPC)]
        k2_pack = [packp.tile([128, S], BF16, tag=f"k2p{h}", name=f"k2p{h}")
                   for h in range(HPC)]
        for h in range(HPC):
            nc.sync.dma_start(out=q_pack[h],
                              in_=packs_dram[h * 128:(h + 1) * 128, :])
            nc.sync.dma_start(out=k1_pack[h],
                              in_=packs_dram[(4 + h) * 128:(5 + h) * 128, :])
            nc.sync.dma_start(out=k2_pack[h],
                              in_=packs_dram[(8 + h) * 128:(9 + h) * 128, :])

        # ---- phase F: attention ------------------------------------------
        with ExitStack() as phase_f:
            dots_pool = phase_f.enter_context(
                tc.tile_pool(name="dots", bufs=5, space="PSUM"))
            tpps = phase_f.enter_context(
                tc.tile_pool(name="tpps", bufs=2, space="PSUM"))
            avps = phase_f.enter_context(
                tc.tile_pool(name="avps", bufs=1, space="PSUM"))
            attn_pool = phase_f.enter_context(tc.tile_pool(name="attn",
                                                           bufs=3))
            attnT_pool = phase_f.enter_context(tc.tile_pool(name="attnT",
                                                            bufs=2))
            small = phase_f.enter_context(tc.tile_pool(name="small", bufs=8))

            for t in range(NT):
                for hh in range(2):
                    h4 = 2 * t + hh
                    for sup in range(SCH // 4):
                        attnT = attnT_pool.tile([128, 4 * S], BF16,
                                                tag="attnT", name="attnT")
                        for u in range(4):
                            ic = sup * 4 + u
                            isl = slice(ic * 128, (ic + 1) * 128)
                            col = slice(2 * ic + hh, 2 * ic + hh + 1)
                            dots = [dots_pool.tile([128, 512], F32,
                                                   tag="dots", name="dots")
                                    for _ in range(NJ)]
                            for jn in range(NJ):
                                jsl = slice(jn * 512, (jn + 1) * 512)
                                nc.tensor.matmul(
                                    dots[jn], q_pack[h4][:, isl],
                                    k1_pack[h4][:, jsl],
                                    start=True, stop=False,
                                    skip_group_check=True)
                                nc.tensor.matmul(
                                    dots[jn], q_pack[h4][:, isl],
                                    k2_pack[h4][:, jsl],
                                    start=False, stop=True,
                                    skip_group_check=True)
                            mx = [small.tile([128, 1], F32, tag=f"mx{j}",
                                             name=f"mx{j}")
                                  for j in range(NJ)]
                            for jn in range(NJ):
                                nc.vector.tensor_reduce(out=mx[jn],
                                                        in_=dots[jn],
                                                        axis=AX.X, op=ALU.max)
                            nc.vector.tensor_max(mx[0], mx[0], mx[1])
                            nc.vector.tensor_max(mx[2], mx[2], mx[3])
                            nc.vector.tensor_max(mx[0], mx[0], mx[2])
                            bias = small.tile([128, 1], F32, tag="bias",
                                              name="bias")
                            nc.vector.tensor_mul(bias, mx[0],
                                                 nrq_all[t][:, col])
                            attn = attn_pool.tile([128, S], BF16, tag="attn",
                                                  name="attn")
                            sm = [small.tile([128, 1], F32, tag=f"sm{j}",
                                             name=f"sm{j}")
                                  for j in range(NJ)]
                            for jn in range(NJ):
                                jsl = slice(jn * 512, (jn + 1) * 512)
                                nc.scalar.activation(
                                    out=attn[:, jsl], in_=dots[jn],
                                    func=AF.Exp, bias=bias,
                                    scale=rq_all[t][:, col],
                                    accum_out=sm[jn])
                            nc.vector.tensor_add(sm[0], sm[0], sm[1])
                            nc.vector.tensor_add(sm[2], sm[2], sm[3])
                            nc.vector.tensor_add(sm[0], sm[0], sm[2])
                            # store 1/sum; normalization deferred to out^T
                            nc.vector.reciprocal(rs_all[t][:, col], sm[0])
                            if DEBUG_DUMPS and t == 0 and hh == 0 and \
                                    sup == 0 and u == 0:
                                nc.sync.dma_start(out=dbg_at[:], in_=attn)
                            for jq in range(SCH // 4):
                                tp = tpps.tile([128, 512], BF16, tag="tp",
                                               name="tp")
                                for j2 in range(4):
                                    jc = jq * 4 + j2
                                    nc.tensor.transpose(
                                        tp[:, j2 * 128:(j2 + 1) * 128],
                                        attn[:, jc * 128:(jc + 1) * 128],
                                        ident)
                                for j2 in range(4):
                                    jc = jq * 4 + j2
                                    dsl2 = slice(jc * 512 + u * 128,
                                                 jc * 512 + u * 128 + 128)
                                    if j2 % 2 == 0:
                                        nc.vector.tensor_copy(
                                            attnT[:, dsl2],
                                            tp[:, j2 * 128:(j2 + 1) * 128])
                                    else:
                                        nc.scalar.copy(
                                            attnT[:, dsl2],
                                            tp[:, j2 * 128:(j2 + 1) * 128])
                        av = avps.tile([64, 512], F32, tag="av", name="av")
                        for jc in range(SCH):
                            nc.tensor.matmul(
                                av, v_nat[jc][:, h4 * 64:(h4 + 1) * 64],
                                attnT[:, jc * 512:(jc + 1) * 512],
                                start=(jc == 0), stop=(jc == SCH - 1))
                        poff = hh * 64
                        ssl = slice(sup * 512, (sup + 1) * 512)
                        if sup % 2 == 0:
                            nc.vector.tensor_copy(outT[t][poff:poff + 64, ssl],
                                                  av)
                        else:
                            nc.scalar.copy(outT[t][poff:poff + 64, ssl], av)

        # ---- phase F2: normalize out^T by 1/sum ---------------------------
        with ExitStack() as phase_f2:
            rowps = phase_f2.enter_context(
                tc.tile_pool(name="rowps", bufs=2, space="PSUM"))
            bcps = phase_f2.enter_context(
                tc.tile_pool(name="bcps", bufs=2, space="PSUM"))
            rrow = phase_f2.enter_context(tc.tile_pool(name="rrow", bufs=2))
            for t in range(NT):
                rowA = rrow.tile([1, S], F32, tag="rowA", name="rowA")
                rowB = rrow.tile([1, S], F32, tag="rowB", name="rowB")
                for hh, row in ((0, rowA), (1, rowB)):
                    for nq in range(NJ):
                        rp = rowps.tile([1, 512], F32, tag="rp", name="rp")
                        for sc4 in range(4):
                            sc = nq * 4 + sc4
                            col = slice(2 * sc + hh, 2 * sc + hh + 1)
                            nc.tensor.transpose(
                                rp[:, sc4 * 128:(sc4 + 1) * 128],
                                rs_all[t][:, col], ident_f)
                        nc.vector.tensor_copy(
                            row[:, nq * 512:(nq + 1) * 512], rp)
                rs2 = rrow.tile([2, S], F32, tag="rs2", name="rs2")
                nc.sync.dma_start(out=rs2[0:1, :], in_=rowA)
                nc.sync.dma_start(out=rs2[1:2, :], in_=rowB)
                for nq in range(NJ):
                    nsl = slice(nq * 512, (nq + 1) * 512)
                    bc = bcps.tile([128, 512], F32, tag="bc", name="bc")
                    nc.tensor.matmul(bc, e2t2, rs2[:, nsl],
                                     start=True, stop=True)
                    nc.vector.tensor_mul(outT[t][:, nsl], outT[t][:, nsl], bc)

        if DEBUG_DUMPS:
            nc.sync.dma_start(out=dbg_ot[:], in_=outT[0])

        # ---- phase G: output projection (f32r) ---------------------------
        with ExitStack() as phase_g:
            wops = phase_g.enter_context(
                tc.tile_pool(name="wops", bufs=4, space="PSUM"))
            gpool = phase_g.enter_context(tc.tile_pool(name="g", bufs=1))
            ostage = phase_g.enter_context(tc.tile_pool(name="ost", bufs=2))

            wo = [gpool.tile([128, DIM], _OF, tag=f"wo{k}", name=f"wo{k}")
                  for k in range(2)]
            for k in range(2):
                nc.sync.dma_start(out=wo[k], in_=wo_d[k * 128:(k + 1) * 128, :])
            for sc in range(SCH):
                ssl = slice(sc * 128, (sc + 1) * 128)
                ost = ostage.tile([128, DIM], F32, tag="ost", name="ost")
                for nn in range(2):
                    nsl = slice(nn * 512, (nn + 1) * 512)
                    ps = wops.tile([128, 512], F32, tag="wops", name="wops")
                    for kk in range(2):
                        nc.tensor.matmul(ps, outT[kk][:, ssl], wo[kk][:, nsl],
                                         start=(kk == 0), stop=(kk == 1))
                    if nn % 2 == 0:
                        nc.vector.tensor_copy(ost[:, nsl], ps)
                    else:
                        nc.scalar.copy(ost[:, nsl], ps)
                nc.sync.dma_start(out=out_d[ssl, :], in_=ost)

    _fix_multiwaits(nc)
    return nc


_NC = None


def _get_nc():
    global _NC
    if _NC is None:
        _NC = _build_program()
    return _NC


def kernel(x, ln_w, ln_b, Wq, Wkv, q_gamma, k_gamma, Wo):
    x = np.asarray(x, np.float32)
    ln_w = np.asarray(ln_w, np.float32)
    ln_b = np.asarray(ln_b, np.float32)
    Wq = np.asarray(Wq, np.float32)
    Wkv = np.asarray(Wkv, np.float32)
    q_gamma = np.asarray(q_gamma, np.float32)
    k_gamma = np.asarray(k_gamma, np.float32)
    Wo = np.asarray(Wo, np.float32)
    Wk_full = Wkv[:, :H * DH]
    Wv_full = Wkv[:, H * DH:]

    bf = ml_dtypes.bfloat16
    e2_host = np.zeros((128, 2), np.float32)
    e2_host[0:64, 0] = 1.0
    e2_host[64:128, 1] = 1.0
    e2t_host = np.ascontiguousarray(e2_host.T)

    def aug_weights(Wsl):
        # [1152, INC]: [ln_w*W; -colsum/1024; ln_b@W; zeros]
        Wt = ln_w[:, None] * Wsl
        out = np.zeros((9 * 128, INC), np.float32)
        out[:DIM] = Wt
        out[DIM] = -Wt.sum(axis=0) / DIM
        out[DIM + 1] = ln_b @ Wsl
        return out

    in_maps = []
    for c in range(NCORES):
        b = c // (NCORES // B)
        g0 = (c % (NCORES // B)) * HPC
        hsl = slice(g0 * DH, (g0 + HPC) * DH)
        xt_host = np.ascontiguousarray(x[b].T)
        in_maps.append({
            "xT": xt_host,
            "xTb": xt_host.astype(bf),
            "Wq": aug_weights(Wq[:, hsl]),
            "Wk": aug_weights(Wk_full[:, hsl]),
            "Wv": aug_weights(Wv_full[:, hsl]).astype(bf),
            "Wo": np.ascontiguousarray(Wo[hsl, :]) if _OF != BF16
                  else np.ascontiguousarray(Wo[hsl, :]).astype(bf),
            "gq": (8.0 * q_gamma[g0:g0 + HPC]).reshape(INC, 1).astype(np.float32),
            "gk": (8.0 * k_gamma[g0:g0 + HPC]).reshape(INC, 1).astype(np.float32),
            "E2": e2_host,
            "E2T": e2t_host,
        })

    res = run_bass_kernel_spmd(_get_nc(), in_maps, list(range(NCORES))).results
    gpb = NCORES // B
    out = np.zeros((B, S, DIM), np.float32)
    for b in range(B):
        acc = np.zeros((S, DIM), np.float32)
        for c in range(b * gpb, (b + 1) * gpb):
            acc += res[c]["out"]
        out[b] = acc
    return out


# revision 43
# speedup vs baseline: 1.1899x; 1.0367x over previous
"""Trainium2 Bass kernel for nn_Attention (dense transformer attention layer).

Sharding: 8 cores = 2 (batch) x 4 (head-group TP).  Core c handles batch
c//4 and heads [4*(c%4), 4*(c%4)+4).  Each core computes LayerNorm (in the
transposed domain, folded into augmented projection matmuls), q/k/v
projections, per-head RMS-norm'd attention, and a partial output
projection; the host sums the 4 partials per batch.

Precision: the dots have sigma~512 (scale=sqrt(dh) applied to BOTH q and k),
so softmax is near-argmax and the q/k path (projections + QK^T) must run in
fp32 on the PE (4 cyc/row).  The v path, attention weights, and Wo run in
bf16.

Layout notes:
 - x is fed transposed (x^T [DIM, S]) so all matmuls contract over
   partitions without on-device transposition of x.
 - LayerNorm: mean/var per token are computed with ones-stationary matmuls
   (bf16 copy of x^T), then folded into the projections via two augmented
   contraction rows: z = [x^T; colsums; 1/r], W = [ln_w*W; -u/1024; ln_b@W].
   The per-token 1/r factor cancels in q/k (RMSNorm scale invariance) and is
   applied explicitly to v.
 - RMSNorm of q is folded into the softmax exp (ACT scale/bias are
   per-partition APs); RMSNorm of k is applied via a broadcast multiply.
"""
import numpy as np
import ml_dtypes
import os
from contextlib import ExitStack

import concourse.bass as bass
import concourse.tile as tile
from concourse import mybir
from concourse.bass_utils import run_bass_kernel_spmd
from concourse.masks import make_identity

F32 = mybir.dt.float32
F32R = mybir.dt.float32r
BF16 = mybir.dt.bfloat16
_OF = F32R if os.environ.get("OUT_F32R", "0") == "1" else F32
AF = mybir.ActivationFunctionType
ALU = mybir.AluOpType
AX = mybir.AxisListType

B, S, DIM, H, DH = 2, 2048, 1024, 16, 64
NCORES = 8
HPC = 4                  # heads per core
INC = HPC * DH           # 256 inner dims per core
KCH = DIM // 128         # 8 contraction chunks of x
NT = 2                   # q/k/v^T tiles per core ([128, S] each, 2 heads per tile)
SCH = S // 128           # 16 token chunks
NJ = S // 512            # 4 moving chunks

_TPB_ENGINES = None


def _fix_multiwaits(nc, max_waits=1):
    """walrus in this container encodes at most one semaphore wait per TPB
    instruction; split extras onto single-wait NoOps ahead of the
    instruction (same engine => program order preserves semantics)."""
    global _TPB_ENGINES
    if _TPB_ENGINES is None:
        _TPB_ENGINES = {
            mybir.EngineType.PE,
            mybir.EngineType.Activation,
            mybir.EngineType.DVE,
            mybir.EngineType.Pool,
            mybir.EngineType.SP,
        }
    nsplit = 0
    for f in nc.m.functions:
        for bb in f.blocks:
            new = []
            for inst in bb.instructions:
                si = inst.sync_info
                if (
                    inst.engine in _TPB_ENGINES
                    and si is not None
                    and si.on_wait
                    and len(si.on_wait) > max_waits
                ):
                    waits = list(si.on_wait)
                    extra, keep = waits[:-max_waits], waits[-max_waits:]
                    for w in extra:
                        nop = mybir.InstNoOp(
                            name=f"I-{nc.next_id()}",
                            ins=[],
                            outs=[],
                            engine=inst.engine,
                            sync_info=mybir.SyncInfo(on_wait=[w], on_update=[]),
                        )
                        try:
                            nc.register_instruction(nop, overwrite=True)
                        except Exception:
                            pass
                        new.append(nop)
                    try:
                        si.on_wait[:] = keep
                    except TypeError:
                        inst.sync_info = mybir.SyncInfo(
                            on_wait=keep, on_update=si.on_update
                        )
                    nsplit += 1
                new.append(inst)
            bb.instructions[:] = new
    return nsplit


DEBUG_DUMPS = False


def _build_program():
    nc = bass.Bass("TRN2", target_bir_lowering=False, debug=False,
                   num_devices=NCORES)
    din = lambda n, s, d: nc.dram_tensor(n, list(s), d, kind="ExternalInput").ap()
    xT = din("xT", (DIM, S), F32)
    xTb = din("xTb", (DIM, S), BF16)
    wq_d = din("Wq", (9 * 128, INC), F32)
    wk_d = din("Wk", (9 * 128, INC), F32)
    wv_d = din("Wv", (9 * 128, INC), BF16)
    wo_d = din("Wo", (INC, DIM), _OF)
    gq_d = din("gq", (INC, 1), F32)
    gk_d = din("gk", (INC, 1), F32)
    e2_d = din("E2", (128, 2), F32)
    e2t_d = din("E2T", (2, 128), F32)
    out_d = nc.dram_tensor("out", [S, DIM], F32, kind="ExternalOutput").ap()
    packs_dram = nc.dram_tensor("packs_scratch", [12 * 128, S], BF16,
                                kind="Internal").ap()
    if DEBUG_DUMPS:
        dbg_q = nc.dram_tensor("dbg_q", [128, S], F32, kind="ExternalOutput").ap()
        dbg_k = nc.dram_tensor("dbg_k", [128, S], F32, kind="ExternalOutput").ap()
        dbg_rq = nc.dram_tensor("dbg_rq", [128, 2 * SCH], F32,
                                kind="ExternalOutput").ap()
        dbg_v = nc.dram_tensor("dbg_v", [128, INC], BF16,
                               kind="ExternalOutput").ap()
        dbg_rc = nc.dram_tensor("dbg_rc", [128, SCH], F32,
                                kind="ExternalOutput").ap()
        dbg_at = nc.dram_tensor("dbg_at", [128, S], BF16,
                                kind="ExternalOutput").ap()
        dbg_ot = nc.dram_tensor("dbg_ot", [128, S], BF16,
                                kind="ExternalOutput").ap()

    with tile.TileContext(nc) as tc, ExitStack() as ctx:
        # ---- long-lived pools
        consts = ctx.enter_context(tc.tile_pool(name="consts", bufs=1))
        vpool = ctx.enter_context(tc.tile_pool(name="v", bufs=1))

        ident = consts.tile([128, 128], BF16, tag="ident", name="ident")
        make_identity(nc, ident)
        ident_f = consts.tile([128, 128], F32, tag="ident_f", name="ident_f")
        make_identity(nc, ident_f)
        ones_b = consts.tile([128, 1], BF16, tag="ones", name="ones")
        nc.vector.memset(ones_b, 1.0)
        e2 = consts.tile([128, 2], F32, tag="e2", name="e2")
        nc.sync.dma_start(out=e2, in_=e2_d[:])
        e2t2 = consts.tile([2, 128], F32, tag="e2t2", name="e2t2")
        nc.sync.dma_start(out=e2t2, in_=e2t_d[:])
        r_col = consts.tile([128, SCH], F32, tag="r_col", name="r_col")
        gq_t = [consts.tile([128, 1], F32, tag=f"gq{t}", name=f"gq{t}")
                for t in range(NT)]
        gk_t = [consts.tile([128, 1], F32, tag=f"gk{t}", name=f"gk{t}")
                for t in range(NT)]
        for t in range(NT):
            nc.sync.dma_start(out=gq_t[t], in_=gq_d[t * 128:(t + 1) * 128, :])
            nc.sync.dma_start(out=gk_t[t], in_=gk_d[t * 128:(t + 1) * 128, :])
        eps5 = consts.tile([128, 1], F32, tag="eps5", name="eps5")
        nc.vector.memset(eps5, 1e-5)
        aug_f = consts.tile([2, S], F32, tag="aug_f", name="aug_f")
        nc.vector.memset(aug_f, 0.0)
        r_sb = consts.tile([1, S], F32, tag="r_sb", name="r_sb")

        # rmsnorm factors for q (per-partition layout; col = 2*sc+hh)
        rq_all = [consts.tile([128, 2 * SCH], F32, tag=f"rq{t}", name=f"rq{t}")
                  for t in range(NT)]
        nrq_all = [consts.tile([128, 2 * SCH], F32, tag=f"nrq{t}", name=f"nrq{t}")
                   for t in range(NT)]
        rs_all = [consts.tile([128, 2 * SCH], F32, tag=f"rs{t}", name=f"rs{t}")
                  for t in range(NT)]

        v_nat = [vpool.tile([128, INC], BF16, tag=f"vn{j}", name=f"vn{j}")
                 for j in range(SCH)]


        with ExitStack() as phase_bcde:
            qkpool = phase_bcde.enter_context(tc.tile_pool(name="qk", bufs=1))
            qhat = [qkpool.tile([128, S], F32, tag=f"qh{t}", name=f"qh{t}")
                    for t in range(NT)]
            khat = [qkpool.tile([128, S], F32, tag=f"kh{t}", name=f"kh{t}")
                    for t in range(NT)]

            with ExitStack() as phase_bc:
                xpool = phase_bc.enter_context(tc.tile_pool(name="x", bufs=1))
                wpool = phase_bc.enter_context(tc.tile_pool(name="w", bufs=1))
                xt = [xpool.tile([128, S], F32, tag=f"xt{k}", name=f"xt{k}")
                      for k in range(KCH)]
                wq = [wpool.tile([128, INC], F32, tag=f"wq{k}", name=f"wq{k}")
                      for k in range(9)]
                wk = [wpool.tile([128, INC], F32, tag=f"wk{k}", name=f"wk{k}")
                      for k in range(9)]

                # ---- phase B: LayerNorm stats (bf16 x^T streamed) ---------
                with ExitStack() as phase_b:
                    xbpool = phase_b.enter_context(
                        tc.tile_pool(name="xb", bufs=2))
                    x2pool = phase_b.enter_context(
                        tc.tile_pool(name="x2", bufs=1))
                    browp = phase_b.enter_context(
                        tc.tile_pool(name="brow", bufs=1))
                    stps = phase_b.enter_context(
                        tc.tile_pool(name="stps", bufs=1, space="PSUM"))
                    sums_ps = stps.tile([1, S], F32, tag="sums", name="sums")
                    sumsq_ps = stps.tile([1, S], F32, tag="sumsq", name="sumsq")
                    for k in range(KCH):
                        nc.sync.dma_start(out=xt[k],
                                          in_=xT[k * 128:(k + 1) * 128, :])
                        xb = xbpool.tile([128, S], BF16, tag="xb", name="xb")
                        nc.sync.dma_start(out=xb,
                                          in_=xTb[k * 128:(k + 1) * 128, :])
                        # x_lo = x - bf16(x): second bf16x2 term so the token
                        # means are fp32-accurate (mu noise shifts dots)
                        xl = x2pool.tile([128, S], BF16, tag="xl", name="xl")
                        nc.vector.tensor_sub(xl, xt[k], xb)
                        x2 = x2pool.tile([128, S], BF16, tag="x2", name="x2")
                        nc.scalar.square(x2, xb)
                        for n in range(NJ):
                            nsl = slice(n * 512, (n + 1) * 512)
                            nc.tensor.matmul(sums_ps[:, nsl], ones_b,
                                             xb[:, nsl],
                                             start=(k == 0), stop=False,
                                             skip_group_check=True)
                            nc.tensor.matmul(sums_ps[:, nsl], ones_b,
                                             xl[:, nsl],
                                             start=False,
                                             stop=(k == KCH - 1),
                                             skip_group_check=True)
                            nc.tensor.matmul(sumsq_ps[:, nsl], ones_b,
                                             x2[:, nsl],
                                             start=(k == 0),
                                             stop=(k == KCH - 1),
                                             skip_group_check=True)
                    for k in range(9):
                        nc.sync.dma_start(out=wq[k],
                                          in_=wq_d[k * 128:(k + 1) * 128, :])
                        nc.sync.dma_start(out=wk[k],
                                          in_=wk_d[k * 128:(k + 1) * 128, :])
                    # b = sumsq - sums^2/1024  (all [1, S])
                    tmp = browp.tile([1, S], F32, tag="strow", name="strow")
                    nc.vector.tensor_scalar_mul(tmp, sums_ps, 1.0 / DIM)
                    nc.vector.tensor_mul(tmp, tmp, sums_ps)
                    nc.vector.tensor_sub(tmp, sumsq_ps, tmp)
                    # t = b/1024 + 1e-5 (= var+eps); r = rsqrt(t) via Newton
                    tv = browp.tile([1, S], F32, tag="tv", name="tv")
                    nc.vector.tensor_scalar(tv, tmp, 1.0 / DIM, 1e-5,
                                            op0=ALU.mult, op1=ALU.add)
                    nc.scalar.activation(out=tmp, in_=tv, func=AF.Sqrt,
                                         bias=0.0, scale=1.0)
                    nc.vector.reciprocal(r_sb, tmp)
                    nc.vector.tensor_mul(tmp, r_sb, r_sb)
                    nc.vector.tensor_mul(tmp, tmp, tv)
                    nc.vector.tensor_scalar(tmp, tmp, -0.5, 1.5,
                                            op0=ALU.mult, op1=ALU.add)
                    nc.vector.tensor_mul(r_sb, r_sb, tmp)
                    # invr = t * r
                    nc.vector.tensor_mul(tv, tv, r_sb)
                    nc.vector.tensor_copy(aug_f[0:1, :], sums_ps)
                    nc.sync.dma_start(out=aug_f[1:2, :], in_=tv)

                # ---- phase C: q/k projections (fp32) ----------------------
                with ExitStack() as phase_c:
                    prps = phase_c.enter_context(
                        tc.tile_pool(name="prps", bufs=4, space="PSUM"))
                    for wt, dst in ((wq, qhat), (wk, khat)):
                        for m in range(NT):
                            msl = slice(m * 128, (m + 1) * 128)
                            for n in range(NJ):
                                nsl = slice(n * 512, (n + 1) * 512)
                                ps = prps.tile([128, 512], F32, tag="proj",
                                               name="proj")
                                for k in range(KCH):
                                    nc.tensor.matmul(ps, wt[k][:, msl],
                                                     xt[k][:, nsl],
                                                     start=(k == 0),
                                                     stop=False)
                                nc.tensor.matmul(ps, wt[8][0:2, msl],
                                                 aug_f[:, nsl],
                                                 start=False, stop=True)
                                if n % 2 == 0:
                                    nc.vector.tensor_copy(dst[m][:, nsl], ps)
                                else:
                                    nc.scalar.copy(dst[m][:, nsl], ps)

            # ---- phase E: v projection (bf16, k-outer) + v transposes ----
            with ExitStack() as phase_e:
                xbpool2 = phase_e.enter_context(tc.tile_pool(name="xb2",
                                                             bufs=2))
                wvpool = phase_e.enter_context(tc.tile_pool(name="wv", bufs=1))
                vtpool = phase_e.enter_context(tc.tile_pool(name="vT", bufs=1))
                phase_e1 = phase_e.enter_context(ExitStack())
                vprps = phase_e1.enter_context(
                    tc.tile_pool(name="vprps", bufs=1, space="PSUM"))
                aug_b = wvpool.tile([2, S], BF16, tag="aug_b", name="aug_b")
                nc.gpsimd.dma_start(out=aug_b, in_=aug_f)
                wv = [wvpool.tile([128, INC], BF16, tag=f"wv{k}",
                                  name=f"wv{k}") for k in range(9)]
                for k in range(9):
                    nc.sync.dma_start(out=wv[k],
                                      in_=wv_d[k * 128:(k + 1) * 128, :])
                vT = [vtpool.tile([128, S], F32, tag=f"vT{t}", name=f"vT{t}")
                      for t in range(NT)]
                vps = [vprps.tile([128, 512], F32, tag=f"vp{i}", name=f"vp{i}")
                       for i in range(NT * NJ)]
                for k in range(KCH):
                    xb = xbpool2.tile([128, S], BF16, tag="xb2", name="xb2")
                    nc.sync.dma_start(out=xb,
                                      in_=xTb[k * 128:(k + 1) * 128, :])
                    for m in range(NT):
                        msl = slice(m * 128, (m + 1) * 128)
                        for n in range(NJ):
                            nsl = slice(n * 512, (n + 1) * 512)
                            nc.tensor.matmul(vps[m * NJ + n], wv[k][:, msl],
                                             xb[:, nsl],
                                             start=(k == 0), stop=False)
                for m in range(NT):
                    msl = slice(m * 128, (m + 1) * 128)
                    for n in range(NJ):
                        nsl = slice(n * 512, (n + 1) * 512)
                        nc.tensor.matmul(vps[m * NJ + n], wv[8][0:2, msl],
                                         aug_b[:, nsl], start=False, stop=True)
                        if n % 2 == 0:
                            nc.vector.tensor_copy(vT[m][:, nsl],
                                                  vps[m * NJ + n])
                        else:
                            nc.scalar.copy(vT[m][:, nsl], vps[m * NJ + n])

                # r_col + v transposes; fold r into the psum->v_nat copy so
                # v is rounded to bf16 exactly once
                phase_e1.close()
                with ExitStack() as phase_e2:
                    vtps = phase_e2.enter_context(
                        tc.tile_pool(name="vtps", bufs=2, space="PSUM"))
                    rtp = phase_e2.enter_context(
                        tc.tile_pool(name="rtp", bufs=2, space="PSUM"))
                    for j in range(SCH):
                        rp = rtp.tile([128, 1], F32, tag="rp", name="rp")
                        nc.tensor.transpose(rp,
                                            r_sb[0:1, j * 128:(j + 1) * 128],
                                            ident_f[0:1, 0:1])
                        nc.vector.tensor_copy(r_col[:, j:j + 1], rp)
                    for t in range(NT):
                        for j in range(SCH):
                            tp = vtps.tile([128, 128], F32, tag="vtp",
                                           name="vtp")
                            nc.tensor.transpose(
                                tp, vT[t][:, j * 128:(j + 1) * 128], ident_f)
                            nc.vector.tensor_scalar_mul(
                                v_nat[j][:, t * 128:(t + 1) * 128], tp,
                                r_col[:, j:j + 1])

            # ---- phase D: rmsnorm factors + bf16x2 packs -----------------
            with ExitStack() as phase_d:
                sq2pool = phase_d.enter_context(
                    tc.tile_pool(name="sq2", bufs=2))
                dstage = phase_d.enter_context(
                    tc.tile_pool(name="dstage", bufs=1))
                ssqps = phase_d.enter_context(
                    tc.tile_pool(name="ssqps", bufs=2, space="PSUM"))
                sskps = phase_d.enter_context(
                    tc.tile_pool(name="sskps", bufs=1, space="PSUM"))
                kmps = phase_d.enter_context(
                    tc.tile_pool(name="kmps", bufs=2, space="PSUM"))
                for t in range(NT):
                    q2 = sq2pool.tile([128, S], F32, tag="q2", name="q2")
                    nc.gpsimd.tensor_mul(q2, qhat[t], qhat[t])
                    for sc in range(SCH):
                        psq = ssqps.tile([128, 2], F32, tag="ssq", name="ssq")
                        nc.tensor.matmul(psq, q2[:, sc * 128:(sc + 1) * 128],
                                         e2, start=True, stop=True)
                        nc.vector.tensor_copy(
                            rq_all[t][:, 2 * sc:2 * sc + 2], psq)
                    # rq = rsqrt(ss/64 + 1e-8), Newton-refined
                    tq = dstage.tile([128, 2 * SCH], F32, tag="tq", name="tq")
                    nc.vector.tensor_scalar(tq, rq_all[t], 1.0 / DH, 1e-8,
                                            op0=ALU.mult, op1=ALU.add)
                    nc.scalar.activation(out=rq_all[t], in_=tq, func=AF.Sqrt,
                                         bias=0.0, scale=1.0)
                    nc.vector.reciprocal(rq_all[t], rq_all[t])
                    zz = dstage.tile([128, 2 * SCH], F32, tag="zz", name="zz")
                    nc.vector.tensor_mul(zz, rq_all[t], rq_all[t])
                    nc.vector.tensor_mul(zz, zz, tq)
                    nc.vector.tensor_scalar(zz, zz, -0.5, 1.5,
                                            op0=ALU.mult, op1=ALU.add)
                    nc.vector.tensor_mul(rq_all[t], rq_all[t], zz)
                    nc.vector.tensor_scalar_mul(nrq_all[t], rq_all[t], -1.0)
                    # gamma*8 on q
                    nc.vector.tensor_scalar_mul(qhat[t], qhat[t], gq_t[t])

                    k2t = sq2pool.tile([128, S], F32, tag="q2", name="k2t")
                    nc.gpsimd.tensor_mul(k2t, khat[t], khat[t])
                    psk = sskps.tile([2, S], F32, tag="ssk", name="ssk")
                    for n in range(NJ):
                        nsl = slice(n * 512, (n + 1) * 512)
                        nc.tensor.matmul(psk[:, nsl], e2, k2t[:, nsl],
                                         start=True, stop=True,
                                         skip_group_check=True)
                    rk2 = dstage.tile([2, S], F32, tag="rk2", name="rk2")
                    tk = dstage.tile([2, S], F32, tag="tk", name="tk")
                    nc.vector.tensor_scalar(tk, psk, 1.0 / DH, 1e-8,
                                            op0=ALU.mult, op1=ALU.add)
                    nc.scalar.activation(out=rk2, in_=tk, func=AF.Sqrt,
                                         bias=0.0, scale=1.0)
                    nc.vector.reciprocal(rk2, rk2)
                    zk = dstage.tile([2, S], F32, tag="zk", name="zk")
                    nc.vector.tensor_mul(zk, rk2, rk2)
                    nc.vector.tensor_mul(zk, zk, tk)
                    nc.vector.tensor_scalar(zk, zk, -0.5, 1.5,
                                            op0=ALU.mult, op1=ALU.add)
                    nc.vector.tensor_mul(rk2, rk2, zk)
                    # gamma*8 on k, then rk broadcast multiply (K=2 matmul)
                    nc.vector.tensor_scalar_mul(khat[t], khat[t], gk_t[t])
                    for n in range(NJ):
                        nsl = slice(n * 512, (n + 1) * 512)
                        km = kmps.tile([128, 512], F32, tag="km", name="km")
                        nc.tensor.matmul(km, e2t2, rk2[:, nsl],
                                         start=True, stop=True)
                        nc.vector.tensor_mul(khat[t][:, nsl],
                                             khat[t][:, nsl], km)

                # bf16x2 packs (hi/lo split at native base) -> DRAM scratch
                lostage = phase_d.enter_context(
                    tc.tile_pool(name="lost", bufs=2))
                for t in range(NT):
                    for hh in range(2):
                        h4 = 2 * t + hh
                        rows = slice(hh * 64, hh * 64 + 64)
                        qb = slice(h4 * 128, h4 * 128 + 64)
                        qb2 = slice(h4 * 128 + 64, h4 * 128 + 128)
                        k1b = slice((4 + h4) * 128, (4 + h4) * 128 + 64)
                        k1b2 = slice((4 + h4) * 128 + 64, (4 + h4) * 128 + 128)
                        k2b = slice((8 + h4) * 128, (8 + h4) * 128 + 64)
                        k2b2 = slice((8 + h4) * 128 + 64, (8 + h4) * 128 + 128)
                        hi = lostage.tile([128, S], BF16, tag="hi", name="hi")
                        lo = lostage.tile([128, S], BF16, tag="lo", name="lo")
                        nc.vector.tensor_copy(hi[rows, :], qhat[t][rows, :])
                        nc.vector.tensor_sub(lo[rows, :], qhat[t][rows, :],
                                             hi[rows, :])
                        nc.sync.dma_start(out=packs_dram[qb, :],
                                          in_=hi[rows, :])
                        nc.sync.dma_start(out=packs_dram[qb2, :],
                                          in_=lo[rows, :])
                        hi2 = lostage.tile([128, S], BF16, tag="hi",
                                           name="hi2")
                        lo2 = lostage.tile([128, S], BF16, tag="lo",
                                           name="lo2")
                        nc.vector.tensor_copy(hi2[rows, :], khat[t][rows, :])
                        nc.vector.tensor_sub(lo2[rows, :], khat[t][rows, :],
                                             hi2[rows, :])
                        nc.sync.dma_start(out=packs_dram[k1b, :],
                                          in_=hi2[rows, :])
                        nc.sync.dma_start(out=packs_dram[k1b2, :],
                                          in_=hi2[rows, :])
                        nc.sync.dma_start(out=packs_dram[k2b, :],
                                          in_=lo2[rows, :])
                        nc.sync.dma_start(out=packs_dram[k2b2, :],
                                          in_=lo2[rows, :])

                if DEBUG_DUMPS:
                    nc.sync.dma_start(out=dbg_q[:], in_=qhat[0])
                    nc.sync.dma_start(out=dbg_k[:], in_=khat[0])
                    nc.sync.dma_start(out=dbg_rq[:], in_=rq_all[0])

        if DEBUG_DUMPS:
            nc.sync.dma_start(out=dbg_v[:], in_=v_nat[0])
            nc.sync.dma_start(out=dbg_rc[:], in_=r_col)

        opool = ctx.enter_context(tc.tile_pool(name="o", bufs=1))
        outT = [opool.tile([128, S], _OF, tag=f"oT{k}", name=f"oT{k}")
                for k in range(2)]
        packp = ctx.enter_context(tc.tile_pool(name="packs", bufs=1))
        q_pack = [packp.tile([128, S], BF16, tag=f"qp{h}", name=f"qp{h}")
                  for h in range(HPC)]
        k1_pack = [packp.tile([128, S], BF16, tag=f"k1p{h}", name=f"k1p{h}")
                   for h in range(HPC)]
        k2_pack = [packp.tile([128, S], BF16, tag=f"k2p{h}", name=f"k2p{h}")
                   for h in range(HPC)]
        for h in range(HPC):
            nc.sync.dma_start(out=q_pack[h],
                              in_=packs_dram[h * 128:(h + 1) * 128, :])
            nc.sync.dma_start(out=k1_pack[h],
                              in_=packs_dram[(4 + h) * 128:(5 + h) * 128, :])
            nc.sync.dma_start(out=k2_pack[h],
                              in_=packs_dram[(8 + h) * 128:(9 + h) * 128, :])

        # ---- phase F: attention ------------------------------------------
        with ExitStack() as phase_f:
            dots_pool = phase_f.enter_context(
                tc.tile_pool(name="dots", bufs=5, space="PSUM"))
            tpps = phase_f.enter_context(
                tc.tile_pool(name="tpps", bufs=2, space="PSUM"))
            avps = phase_f.enter_context(
                tc.tile_pool(name="avps", bufs=1, space="PSUM"))
            attn_pool = phase_f.enter_context(tc.tile_pool(name="attn",
                                                           bufs=3))
            attnT_pool = phase_f.enter_context(tc.tile_pool(name="attnT",
                                                            bufs=2))
            small = phase_f.enter_context(tc.tile_pool(name="small", bufs=8))

            for t in range(NT):
                for hh in range(2):
                    h4 = 2 * t + hh
                    for sup in range(SCH // 4):
                        attnT = attnT_pool.tile([128, 4 * S], BF16,
                                                tag="attnT", name="attnT")
                        for u in range(4):
                            ic = sup * 4 + u
                            isl = slice(ic * 128, (ic + 1) * 128)
                            col = slice(2 * ic + hh, 2 * ic + hh + 1)
                            dots = [dots_pool.tile([128, 512], F32,
                                                   tag="dots", name="dots")
                                    for _ in range(NJ)]
                            for jn in range(NJ):
                                jsl = slice(jn * 512, (jn + 1) * 512)
                                nc.tensor.matmul(
                                    dots[jn], q_pack[h4][:, isl],
                                    k1_pack[h4][:, jsl],
                                    start=True, stop=False,
                                    skip_group_check=True)
                                nc.tensor.matmul(
                                    dots[jn], q_pack[h4][:, isl],
                                    k2_pack[h4][:, jsl],
                                    start=False, stop=True,
                                    skip_group_check=True)
                            mx = [small.tile([128, 1], F32, tag=f"mx{j}",
                                             name=f"mx{j}")
                                  for j in range(NJ)]
                            for jn in range(NJ):
                                nc.vector.tensor_reduce(out=mx[jn],
                                                        in_=dots[jn],
                                                        axis=AX.X, op=ALU.max)
                            nc.vector.tensor_max(mx[0], mx[0], mx[1])
                            nc.vector.tensor_max(mx[2], mx[2], mx[3])
                            nc.vector.tensor_max(mx[0], mx[0], mx[2])
                            bias = small.tile([128, 1], F32, tag="bias",
                                              name="bias")
                            nc.vector.tensor_mul(bias, mx[0],
                                                 nrq_all[t][:, col])
                            attn = attn_pool.tile([128, S], BF16, tag="attn",
                                                  name="attn")
                            sm = [small.tile([128, 1], F32, tag=f"sm{j}",
                                             name=f"sm{j}")
                                  for j in range(NJ)]
                            for jn in range(NJ):
                                jsl = slice(jn * 512, (jn + 1) * 512)
                                nc.scalar.activation(
                                    out=attn[:, jsl], in_=dots[jn],
                                    func=AF.Exp, bias=bias,
                                    scale=rq_all[t][:, col],
                                    accum_out=sm[jn])
                            nc.vector.tensor_add(sm[0], sm[0], sm[1])
                            nc.vector.tensor_add(sm[2], sm[2], sm[3])
                            nc.vector.tensor_add(sm[0], sm[0], sm[2])
                            # store 1/sum; normalization deferred to out^T
                            nc.vector.reciprocal(rs_all[t][:, col], sm[0])
                            if DEBUG_DUMPS and t == 0 and hh == 0 and \
                                    sup == 0 and u == 0:
                                nc.sync.dma_start(out=dbg_at[:], in_=attn)
                            for jq in range(SCH // 8):
                                tp = tpps.tile([128, 1024], BF16, tag="tp",
                                               name="tp")
                                for j2 in range(8):
                                    jc = jq * 8 + j2
                                    nc.tensor.transpose(
                                        tp[:, j2 * 128:(j2 + 1) * 128],
                                        attn[:, jc * 128:(jc + 1) * 128],
                                        ident)
                                # one strided copy per staging tile: the 8
                                # blocks land at jc*512 + u*128 in attnT
                                eng = nc.vector if jq % 2 == 0 else nc.scalar
                                src = tp[:].rearrange("p (b c) -> p b c", b=8)
                                dst = bass.AP(
                                    tensor=attnT.tensor,
                                    offset=attnT.offset
                                    + jq * 8 * 512 + u * 128,
                                    ap=[attnT.ap[0], [512, 8], [1, 128]],
                                )
                                if jq % 2 == 0:
                                    nc.vector.tensor_copy(dst, src)
                                else:
                                    nc.scalar.copy(dst, src)
                        av = avps.tile([64, 512], F32, tag="av", name="av")
                        for jc in range(SCH):
                            nc.tensor.matmul(
                                av, v_nat[jc][:, h4 * 64:(h4 + 1) * 64],
                                attnT[:, jc * 512:(jc + 1) * 512],
                                start=(jc == 0), stop=(jc == SCH - 1))
                        poff = hh * 64
                        ssl = slice(sup * 512, (sup + 1) * 512)
                        if sup % 2 == 0:
                            nc.vector.tensor_copy(outT[t][poff:poff + 64, ssl],
                                                  av)
                        else:
                            nc.scalar.copy(outT[t][poff:poff + 64, ssl], av)

        # ---- phase F2: normalize out^T by 1/sum ---------------------------
        with ExitStack() as phase_f2:
            rowps = phase_f2.enter_context(
                tc.tile_pool(name="rowps", bufs=2, space="PSUM"))
            bcps = phase_f2.enter_context(
                tc.tile_pool(name="bcps", bufs=2, space="PSUM"))
            rrow = phase_f2.enter_context(tc.tile_pool(name="rrow", bufs=2))
            for t in range(NT):
                rowA = rrow.tile([1, S], F32, tag="rowA", name="rowA")
                rowB = rrow.tile([1, S], F32, tag="rowB", name="rowB")
                for hh, row in ((0, rowA), (1, rowB)):
                    for nq in range(NJ):
                        rp = rowps.tile([1, 512], F32, tag="rp", name="rp")
                        for sc4 in range(4):
                            sc = nq * 4 + sc4
                            col = slice(2 * sc + hh, 2 * sc + hh + 1)
                            nc.tensor.transpose(
                                rp[:, sc4 * 128:(sc4 + 1) * 128],
                                rs_all[t][:, col], ident_f)
                        nc.vector.tensor_copy(
                            row[:, nq * 512:(nq + 1) * 512], rp)
                rs2 = rrow.tile([2, S], F32, tag="rs2", name="rs2")
                nc.sync.dma_start(out=rs2[0:1, :], in_=rowA)
                nc.sync.dma_start(out=rs2[1:2, :], in_=rowB)
                for nq in range(NJ):
                    nsl = slice(nq * 512, (nq + 1) * 512)
                    bc = bcps.tile([128, 512], F32, tag="bc", name="bc")
                    nc.tensor.matmul(bc, e2t2, rs2[:, nsl],
                                     start=True, stop=True)
                    nc.vector.tensor_mul(outT[t][:, nsl], outT[t][:, nsl], bc)

        if DEBUG_DUMPS:
            nc.sync.dma_start(out=dbg_ot[:], in_=outT[0])

        # ---- phase G: output projection (f32r) ---------------------------
        with ExitStack() as phase_g:
            wops = phase_g.enter_context(
                tc.tile_pool(name="wops", bufs=4, space="PSUM"))
            gpool = phase_g.enter_context(tc.tile_pool(name="g", bufs=1))
            ostage = phase_g.enter_context(tc.tile_pool(name="ost", bufs=2))

            wo = [gpool.tile([128, DIM], _OF, tag=f"wo{k}", name=f"wo{k}")
                  for k in range(2)]
            for k in range(2):
                nc.sync.dma_start(out=wo[k], in_=wo_d[k * 128:(k + 1) * 128, :])
            for sc in range(SCH):
                ssl = slice(sc * 128, (sc + 1) * 128)
                ost = ostage.tile([128, DIM], F32, tag="ost", name="ost")
                for nn in range(2):
                    nsl = slice(nn * 512, (nn + 1) * 512)
                    ps = wops.tile([128, 512], F32, tag="wops", name="wops")
                    for kk in range(2):
                        nc.tensor.matmul(ps, outT[kk][:, ssl], wo[kk][:, nsl],
                                         start=(kk == 0), stop=(kk == 1))
                    if nn % 2 == 0:
                        nc.vector.tensor_copy(ost[:, nsl], ps)
                    else:
                        nc.scalar.copy(ost[:, nsl], ps)
                nc.sync.dma_start(out=out_d[ssl, :], in_=ost)

    _fix_multiwaits(nc)
    return nc


_NC = None


def _get_nc():
    global _NC
    if _NC is None:
        _NC = _build_program()
    return _NC


def kernel(x, ln_w, ln_b, Wq, Wkv, q_gamma, k_gamma, Wo):
    x = np.asarray(x, np.float32)
    ln_w = np.asarray(ln_w, np.float32)
    ln_b = np.asarray(ln_b, np.float32)
    Wq = np.asarray(Wq, np.float32)
    Wkv = np.asarray(Wkv, np.float32)
    q_gamma = np.asarray(q_gamma, np.float32)
    k_gamma = np.asarray(k_gamma, np.float32)
    Wo = np.asarray(Wo, np.float32)
    Wk_full = Wkv[:, :H * DH]
    Wv_full = Wkv[:, H * DH:]

    bf = ml_dtypes.bfloat16
    e2_host = np.zeros((128, 2), np.float32)
    e2_host[0:64, 0] = 1.0
    e2_host[64:128, 1] = 1.0
    e2t_host = np.ascontiguousarray(e2_host.T)

    def aug_weights(Wsl):
        # [1152, INC]: [ln_w*W; -colsum/1024; ln_b@W; zeros]
        Wt = ln_w[:, None] * Wsl
        out = np.zeros((9 * 128, INC), np.float32)
        out[:DIM] = Wt
        out[DIM] = -Wt.sum(axis=0) / DIM
        out[DIM + 1] = ln_b @ Wsl
        return out

    in_maps = []
    for c in range(NCORES):
        b = c // (NCORES // B)
        g0 = (c % (NCORES // B)) * HPC
        hsl = slice(g0 * DH, (g0 + HPC) * DH)
        xt_host = np.ascontiguousarray(x[b].T)
        in_maps.append({
            "xT": xt_host,
            "xTb": xt_host.astype(bf),
            "Wq": aug_weights(Wq[:, hsl]),
            "Wk": aug_weights(Wk_full[:, hsl]),
            "Wv": aug_weights(Wv_full[:, hsl]).astype(bf),
            "Wo": np.ascontiguousarray(Wo[hsl, :]) if _OF != BF16
                  else np.ascontiguousarray(Wo[hsl, :]).astype(bf),
            "gq": (8.0 * q_gamma[g0:g0 + HPC]).reshape(INC, 1).astype(np.float32),
            "gk": (8.0 * k_gamma[g0:g0 + HPC]).reshape(INC, 1).astype(np.float32),
            "E2": e2_host,
            "E2T": e2t_host,
        })

    res = run_bass_kernel_spmd(_get_nc(), in_maps, list(range(NCORES))).results
    gpb = NCORES // B
    out = np.zeros((B, S, DIM), np.float32)
    for b in range(B):
        acc = np.zeros((S, DIM), np.float32)
        for c in range(b * gpb, (b + 1) * gpb):
            acc += res[c]["out"]
        out[b] = acc
    return out


# revision 44
# speedup vs baseline: 1.2186x; 1.0241x over previous
"""Trainium2 Bass kernel for nn_Attention (dense transformer attention layer).

Sharding: 8 cores = 2 (batch) x 4 (head-group TP).  Core c handles batch
c//4 and heads [4*(c%4), 4*(c%4)+4).  Each core computes LayerNorm (in the
transposed domain, folded into augmented projection matmuls), q/k/v
projections, per-head RMS-norm'd attention, and a partial output
projection; the host sums the 4 partials per batch.

Precision: the dots have sigma~512 (scale=sqrt(dh) applied to BOTH q and k),
so softmax is near-argmax and the q/k path (projections + QK^T) must run in
fp32 on the PE (4 cyc/row).  The v path, attention weights, and Wo run in
bf16.

Layout notes:
 - x is fed transposed (x^T [DIM, S]) so all matmuls contract over
   partitions without on-device transposition of x.
 - LayerNorm: mean/var per token are computed with ones-stationary matmuls
   (bf16 copy of x^T), then folded into the projections via two augmented
   contraction rows: z = [x^T; colsums; 1/r], W = [ln_w*W; -u/1024; ln_b@W].
   The per-token 1/r factor cancels in q/k (RMSNorm scale invariance) and is
   applied explicitly to v.
 - RMSNorm of q is folded into the softmax exp (ACT scale/bias are
   per-partition APs); RMSNorm of k is applied via a broadcast multiply.
"""
import numpy as np
import ml_dtypes
import os
from contextlib import ExitStack

import concourse.bass as bass
import concourse.tile as tile
from concourse import mybir
from concourse.bass_utils import run_bass_kernel_spmd
from concourse.masks import make_identity

F32 = mybir.dt.float32
F32R = mybir.dt.float32r
BF16 = mybir.dt.bfloat16
_OF = F32R if os.environ.get("OUT_F32R", "0") == "1" else F32
AF = mybir.ActivationFunctionType
ALU = mybir.AluOpType
AX = mybir.AxisListType

B, S, DIM, H, DH = 2, 2048, 1024, 16, 64
NCORES = 8
HPC = 4                  # heads per core
INC = HPC * DH           # 256 inner dims per core
KCH = DIM // 128         # 8 contraction chunks of x
NT = 2                   # q/k/v^T tiles per core ([128, S] each, 2 heads per tile)
SCH = S // 128           # 16 token chunks
NJ = S // 512            # 4 moving chunks

_TPB_ENGINES = None


def _fix_multiwaits(nc, max_waits=1):
    """walrus in this container encodes at most one semaphore wait per TPB
    instruction; split extras onto single-wait NoOps ahead of the
    instruction (same engine => program order preserves semantics)."""
    global _TPB_ENGINES
    if _TPB_ENGINES is None:
        _TPB_ENGINES = {
            mybir.EngineType.PE,
            mybir.EngineType.Activation,
            mybir.EngineType.DVE,
            mybir.EngineType.Pool,
            mybir.EngineType.SP,
        }
    nsplit = 0
    for f in nc.m.functions:
        for bb in f.blocks:
            new = []
            for inst in bb.instructions:
                si = inst.sync_info
                if (
                    inst.engine in _TPB_ENGINES
                    and si is not None
                    and si.on_wait
                    and len(si.on_wait) > max_waits
                ):
                    waits = list(si.on_wait)
                    extra, keep = waits[:-max_waits], waits[-max_waits:]
                    for w in extra:
                        nop = mybir.InstNoOp(
                            name=f"I-{nc.next_id()}",
                            ins=[],
                            outs=[],
                            engine=inst.engine,
                            sync_info=mybir.SyncInfo(on_wait=[w], on_update=[]),
                        )
                        try:
                            nc.register_instruction(nop, overwrite=True)
                        except Exception:
                            pass
                        new.append(nop)
                    try:
                        si.on_wait[:] = keep
                    except TypeError:
                        inst.sync_info = mybir.SyncInfo(
                            on_wait=keep, on_update=si.on_update
                        )
                    nsplit += 1
                new.append(inst)
            bb.instructions[:] = new
    return nsplit


DEBUG_DUMPS = False


def _build_program():
    nc = bass.Bass("TRN2", target_bir_lowering=False, debug=False,
                   num_devices=NCORES)
    din = lambda n, s, d: nc.dram_tensor(n, list(s), d, kind="ExternalInput").ap()
    xT = din("xT", (DIM, S), F32)
    wq_d = din("Wq", (9 * 128, INC), F32)
    wk_d = din("Wk", (9 * 128, INC), F32)
    wv_d = din("Wv", (9 * 128, INC), BF16)
    wo_d = din("Wo", (INC, DIM), _OF)
    gq_d = din("gq", (INC, 1), F32)
    gk_d = din("gk", (INC, 1), F32)
    e2_d = din("E2", (128, 2), F32)
    e2t_d = din("E2T", (2, 128), F32)
    out_d = nc.dram_tensor("out", [S, DIM], F32, kind="ExternalOutput").ap()
    packs_dram = nc.dram_tensor("packs_scratch", [12 * 128, S], BF16,
                                kind="Internal").ap()
    if DEBUG_DUMPS:
        dbg_q = nc.dram_tensor("dbg_q", [128, S], F32, kind="ExternalOutput").ap()
        dbg_k = nc.dram_tensor("dbg_k", [128, S], F32, kind="ExternalOutput").ap()
        dbg_rq = nc.dram_tensor("dbg_rq", [128, 2 * SCH], F32,
                                kind="ExternalOutput").ap()
        dbg_v = nc.dram_tensor("dbg_v", [128, INC], BF16,
                               kind="ExternalOutput").ap()
        dbg_rc = nc.dram_tensor("dbg_rc", [128, SCH], F32,
                                kind="ExternalOutput").ap()
        dbg_at = nc.dram_tensor("dbg_at", [128, S], BF16,
                                kind="ExternalOutput").ap()
        dbg_ot = nc.dram_tensor("dbg_ot", [128, S], BF16,
                                kind="ExternalOutput").ap()

    with tile.TileContext(nc) as tc, ExitStack() as ctx:
        # ---- long-lived pools
        consts = ctx.enter_context(tc.tile_pool(name="consts", bufs=1))
        vpool = ctx.enter_context(tc.tile_pool(name="v", bufs=1))

        ident = consts.tile([128, 128], BF16, tag="ident", name="ident")
        make_identity(nc, ident)
        ident_f = consts.tile([128, 128], F32, tag="ident_f", name="ident_f")
        make_identity(nc, ident_f)
        ones_b = consts.tile([128, 1], BF16, tag="ones", name="ones")
        nc.vector.memset(ones_b, 1.0)
        e2 = consts.tile([128, 2], F32, tag="e2", name="e2")
        nc.sync.dma_start(out=e2, in_=e2_d[:])
        e2t2 = consts.tile([2, 128], F32, tag="e2t2", name="e2t2")
        nc.sync.dma_start(out=e2t2, in_=e2t_d[:])
        r_col = consts.tile([128, SCH], F32, tag="r_col", name="r_col")
        gq_t = [consts.tile([128, 1], F32, tag=f"gq{t}", name=f"gq{t}")
                for t in range(NT)]
        gk_t = [consts.tile([128, 1], F32, tag=f"gk{t}", name=f"gk{t}")
                for t in range(NT)]
        for t in range(NT):
            nc.sync.dma_start(out=gq_t[t], in_=gq_d[t * 128:(t + 1) * 128, :])
            nc.sync.dma_start(out=gk_t[t], in_=gk_d[t * 128:(t + 1) * 128, :])
        eps5 = consts.tile([128, 1], F32, tag="eps5", name="eps5")
        nc.vector.memset(eps5, 1e-5)
        aug_f = consts.tile([2, S], F32, tag="aug_f", name="aug_f")
        nc.vector.memset(aug_f, 0.0)
        r_sb = consts.tile([1, S], F32, tag="r_sb", name="r_sb")

        # rmsnorm factors for q (per-partition layout; col = 2*sc+hh)
        rq_all = [consts.tile([128, 2 * SCH], F32, tag=f"rq{t}", name=f"rq{t}")
                  for t in range(NT)]
        nrq_all = [consts.tile([128, 2 * SCH], F32, tag=f"nrq{t}", name=f"nrq{t}")
                   for t in range(NT)]
        rs_all = [consts.tile([128, 2 * SCH], F32, tag=f"rs{t}", name=f"rs{t}")
                  for t in range(NT)]

        v_nat = [vpool.tile([128, INC], BF16, tag=f"vn{j}", name=f"vn{j}")
                 for j in range(SCH)]


        with ExitStack() as phase_bcde:
            qkpool = phase_bcde.enter_context(tc.tile_pool(name="qk", bufs=1))
            qhat = [qkpool.tile([128, S], F32, tag=f"qh{t}", name=f"qh{t}")
                    for t in range(NT)]
            khat = [qkpool.tile([128, S], F32, tag=f"kh{t}", name=f"kh{t}")
                    for t in range(NT)]

            xpool = phase_bcde.enter_context(tc.tile_pool(name="x", bufs=1))
            xt = [xpool.tile([128, S], F32, tag=f"xt{k}", name=f"xt{k}")
                  for k in range(KCH)]
            with ExitStack() as phase_bc:
                wpool = phase_bc.enter_context(tc.tile_pool(name="w", bufs=1))
                wq = [wpool.tile([128, INC], F32, tag=f"wq{k}", name=f"wq{k}")
                      for k in range(9)]
                wk = [wpool.tile([128, INC], F32, tag=f"wk{k}", name=f"wk{k}")
                      for k in range(9)]

                # ---- phase B: LayerNorm stats (bf16 x^T streamed) ---------
                with ExitStack() as phase_b:
                    xbpool = phase_b.enter_context(
                        tc.tile_pool(name="xb", bufs=2))
                    x2pool = phase_b.enter_context(
                        tc.tile_pool(name="x2", bufs=1))
                    browp = phase_b.enter_context(
                        tc.tile_pool(name="brow", bufs=1))
                    stps = phase_b.enter_context(
                        tc.tile_pool(name="stps", bufs=1, space="PSUM"))
                    sums_ps = stps.tile([1, S], F32, tag="sums", name="sums")
                    sumsq_ps = stps.tile([1, S], F32, tag="sumsq", name="sumsq")
                    for k in range(KCH):
                        nc.sync.dma_start(out=xt[k],
                                          in_=xT[k * 128:(k + 1) * 128, :])
                        xb = xbpool.tile([128, S], BF16, tag="xb", name="xb")
                        nc.vector.tensor_copy(xb, xt[k])
                        # x_lo = x - bf16(x): second bf16x2 term so the token
                        # means are fp32-accurate (mu noise shifts dots)
                        xl = x2pool.tile([128, S], BF16, tag="xl", name="xl")
                        nc.vector.tensor_sub(xl, xt[k], xb)
                        x2 = x2pool.tile([128, S], BF16, tag="x2", name="x2")
                        nc.scalar.square(x2, xb)
                        for n in range(NJ):
                            nsl = slice(n * 512, (n + 1) * 512)
                            nc.tensor.matmul(sums_ps[:, nsl], ones_b,
                                             xb[:, nsl],
                                             start=(k == 0), stop=False,
                                             skip_group_check=True)
                            nc.tensor.matmul(sums_ps[:, nsl], ones_b,
                                             xl[:, nsl],
                                             start=False,
                                             stop=(k == KCH - 1),
                                             skip_group_check=True)
                            nc.tensor.matmul(sumsq_ps[:, nsl], ones_b,
                                             x2[:, nsl],
                                             start=(k == 0),
                                             stop=(k == KCH - 1),
                                             skip_group_check=True)
                    for k in range(9):
                        nc.sync.dma_start(out=wq[k],
                                          in_=wq_d[k * 128:(k + 1) * 128, :])
                        nc.sync.dma_start(out=wk[k],
                                          in_=wk_d[k * 128:(k + 1) * 128, :])
                    # b = sumsq - sums^2/1024  (all [1, S])
                    tmp = browp.tile([1, S], F32, tag="strow", name="strow")
                    nc.vector.tensor_scalar_mul(tmp, sums_ps, 1.0 / DIM)
                    nc.vector.tensor_mul(tmp, tmp, sums_ps)
                    nc.vector.tensor_sub(tmp, sumsq_ps, tmp)
                    # t = b/1024 + 1e-5 (= var+eps); r = rsqrt(t) via Newton
                    tv = browp.tile([1, S], F32, tag="tv", name="tv")
                    nc.vector.tensor_scalar(tv, tmp, 1.0 / DIM, 1e-5,
                                            op0=ALU.mult, op1=ALU.add)
                    nc.scalar.activation(out=tmp, in_=tv, func=AF.Sqrt,
                                         bias=0.0, scale=1.0)
                    nc.vector.reciprocal(r_sb, tmp)
                    nc.vector.tensor_mul(tmp, r_sb, r_sb)
                    nc.vector.tensor_mul(tmp, tmp, tv)
                    nc.vector.tensor_scalar(tmp, tmp, -0.5, 1.5,
                                            op0=ALU.mult, op1=ALU.add)
                    nc.vector.tensor_mul(r_sb, r_sb, tmp)
                    # invr = t * r
                    nc.vector.tensor_mul(tv, tv, r_sb)
                    nc.vector.tensor_copy(aug_f[0:1, :], sums_ps)
                    nc.sync.dma_start(out=aug_f[1:2, :], in_=tv)

                # ---- phase C: q/k projections (fp32) ----------------------
                with ExitStack() as phase_c:
                    prps = phase_c.enter_context(
                        tc.tile_pool(name="prps", bufs=4, space="PSUM"))
                    for wt, dst in ((wq, qhat), (wk, khat)):
                        for m in range(NT):
                            msl = slice(m * 128, (m + 1) * 128)
                            for n in range(NJ):
                                nsl = slice(n * 512, (n + 1) * 512)
                                ps = prps.tile([128, 512], F32, tag="proj",
                                               name="proj")
                                for k in range(KCH):
                                    nc.tensor.matmul(ps, wt[k][:, msl],
                                                     xt[k][:, nsl],
                                                     start=(k == 0),
                                                     stop=False)
                                nc.tensor.matmul(ps, wt[8][0:2, msl],
                                                 aug_f[:, nsl],
                                                 start=False, stop=True)
                                if n % 2 == 0:
                                    nc.vector.tensor_copy(dst[m][:, nsl], ps)
                                else:
                                    nc.scalar.copy(dst[m][:, nsl], ps)

            # ---- phase E: v projection (bf16, k-outer) + v transposes ----
            with ExitStack() as phase_e:
                xbpool2 = phase_e.enter_context(tc.tile_pool(name="xb2",
                                                             bufs=2))
                wvpool = phase_e.enter_context(tc.tile_pool(name="wv", bufs=1))
                vtpool = phase_e.enter_context(tc.tile_pool(name="vT", bufs=1))
                phase_e1 = phase_e.enter_context(ExitStack())
                vprps = phase_e1.enter_context(
                    tc.tile_pool(name="vprps", bufs=1, space="PSUM"))
                aug_b = wvpool.tile([2, S], BF16, tag="aug_b", name="aug_b")
                nc.gpsimd.dma_start(out=aug_b, in_=aug_f)
                wv = [wvpool.tile([128, INC], BF16, tag=f"wv{k}",
                                  name=f"wv{k}") for k in range(9)]
                for k in range(9):
                    nc.sync.dma_start(out=wv[k],
                                      in_=wv_d[k * 128:(k + 1) * 128, :])
                vT = [vtpool.tile([128, S], F32, tag=f"vT{t}", name=f"vT{t}")
                      for t in range(NT)]
                vps = [vprps.tile([128, 512], F32, tag=f"vp{i}", name=f"vp{i}")
                       for i in range(NT * NJ)]
                for k in range(KCH):
                    xb = xbpool2.tile([128, S], BF16, tag="xb2", name="xb2")
                    nc.vector.tensor_copy(xb, xt[k])
                    for m in range(NT):
                        msl = slice(m * 128, (m + 1) * 128)
                        for n in range(NJ):
                            nsl = slice(n * 512, (n + 1) * 512)
                            nc.tensor.matmul(vps[m * NJ + n], wv[k][:, msl],
                                             xb[:, nsl],
                                             start=(k == 0), stop=False)
                for m in range(NT):
                    msl = slice(m * 128, (m + 1) * 128)
                    for n in range(NJ):
                        nsl = slice(n * 512, (n + 1) * 512)
                        nc.tensor.matmul(vps[m * NJ + n], wv[8][0:2, msl],
                                         aug_b[:, nsl], start=False, stop=True)
                        if n % 2 == 0:
                            nc.vector.tensor_copy(vT[m][:, nsl],
                                                  vps[m * NJ + n])
                        else:
                            nc.scalar.copy(vT[m][:, nsl], vps[m * NJ + n])

                # r_col + v transposes; fold r into the psum->v_nat copy so
                # v is rounded to bf16 exactly once
                phase_e1.close()
                with ExitStack() as phase_e2:
                    vtps = phase_e2.enter_context(
                        tc.tile_pool(name="vtps", bufs=2, space="PSUM"))
                    rtp = phase_e2.enter_context(
                        tc.tile_pool(name="rtp", bufs=2, space="PSUM"))
                    for j in range(SCH):
                        rp = rtp.tile([128, 1], F32, tag="rp", name="rp")
                        nc.tensor.transpose(rp,
                                            r_sb[0:1, j * 128:(j + 1) * 128],
                                            ident_f[0:1, 0:1])
                        nc.vector.tensor_copy(r_col[:, j:j + 1], rp)
                    for t in range(NT):
                        for j in range(SCH):
                            tp = vtps.tile([128, 128], F32, tag="vtp",
                                           name="vtp")
                            nc.tensor.transpose(
                                tp, vT[t][:, j * 128:(j + 1) * 128], ident_f)
                            nc.vector.tensor_scalar_mul(
                                v_nat[j][:, t * 128:(t + 1) * 128], tp,
                                r_col[:, j:j + 1])

            # ---- phase D: rmsnorm factors + bf16x2 packs -----------------
            with ExitStack() as phase_d:
                sq2pool = phase_d.enter_context(
                    tc.tile_pool(name="sq2", bufs=2))
                dstage = phase_d.enter_context(
                    tc.tile_pool(name="dstage", bufs=1))
                ssqps = phase_d.enter_context(
                    tc.tile_pool(name="ssqps", bufs=2, space="PSUM"))
                sskps = phase_d.enter_context(
                    tc.tile_pool(name="sskps", bufs=1, space="PSUM"))
                kmps = phase_d.enter_context(
                    tc.tile_pool(name="kmps", bufs=2, space="PSUM"))
                for t in range(NT):
                    q2 = sq2pool.tile([128, S], F32, tag="q2", name="q2")
                    nc.gpsimd.tensor_mul(q2, qhat[t], qhat[t])
                    for sc in range(SCH):
                        psq = ssqps.tile([128, 2], F32, tag="ssq", name="ssq")
                        nc.tensor.matmul(psq, q2[:, sc * 128:(sc + 1) * 128],
                                         e2, start=True, stop=True)
                        nc.vector.tensor_copy(
                            rq_all[t][:, 2 * sc:2 * sc + 2], psq)
                    # rq = rsqrt(ss/64 + 1e-8), Newton-refined
                    tq = dstage.tile([128, 2 * SCH], F32, tag="tq", name="tq")
                    nc.vector.tensor_scalar(tq, rq_all[t], 1.0 / DH, 1e-8,
                                            op0=ALU.mult, op1=ALU.add)
                    nc.scalar.activation(out=rq_all[t], in_=tq, func=AF.Sqrt,
                                         bias=0.0, scale=1.0)
                    nc.vector.reciprocal(rq_all[t], rq_all[t])
                    zz = dstage.tile([128, 2 * SCH], F32, tag="zz", name="zz")
                    nc.vector.tensor_mul(zz, rq_all[t], rq_all[t])
                    nc.vector.tensor_mul(zz, zz, tq)
                    nc.vector.tensor_scalar(zz, zz, -0.5, 1.5,
                                            op0=ALU.mult, op1=ALU.add)
                    nc.vector.tensor_mul(rq_all[t], rq_all[t], zz)
                    nc.vector.tensor_scalar_mul(nrq_all[t], rq_all[t], -1.0)
                    # gamma*8 on q
                    nc.vector.tensor_scalar_mul(qhat[t], qhat[t], gq_t[t])

                    k2t = sq2pool.tile([128, S], F32, tag="q2", name="k2t")
                    nc.gpsimd.tensor_mul(k2t, khat[t], khat[t])
                    psk = sskps.tile([2, S], F32, tag="ssk", name="ssk")
                    for n in range(NJ):
                        nsl = slice(n * 512, (n + 1) * 512)
                        nc.tensor.matmul(psk[:, nsl], e2, k2t[:, nsl],
                                         start=True, stop=True,
                                         skip_group_check=True)
                    rk2 = dstage.tile([2, S], F32, tag="rk2", name="rk2")
                    tk = dstage.tile([2, S], F32, tag="tk", name="tk")
                    nc.vector.tensor_scalar(tk, psk, 1.0 / DH, 1e-8,
                                            op0=ALU.mult, op1=ALU.add)
                    nc.scalar.activation(out=rk2, in_=tk, func=AF.Sqrt,
                                         bias=0.0, scale=1.0)
                    nc.vector.reciprocal(rk2, rk2)
                    zk = dstage.tile([2, S], F32, tag="zk", name="zk")
                    nc.vector.tensor_mul(zk, rk2, rk2)
                    nc.vector.tensor_mul(zk, zk, tk)
                    nc.vector.tensor_scalar(zk, zk, -0.5, 1.5,
                                            op0=ALU.mult, op1=ALU.add)
                    nc.vector.tensor_mul(rk2, rk2, zk)
                    # gamma*8 on k, then rk broadcast multiply (K=2 matmul)
                    nc.vector.tensor_scalar_mul(khat[t], khat[t], gk_t[t])
                    for n in range(NJ):
                        nsl = slice(n * 512, (n + 1) * 512)
                        km = kmps.tile([128, 512], F32, tag="km", name="km")
                        nc.tensor.matmul(km, e2t2, rk2[:, nsl],
                                         start=True, stop=True)
                        nc.vector.tensor_mul(khat[t][:, nsl],
                                             khat[t][:, nsl], km)

                # bf16x2 packs (hi/lo split at native base) -> DRAM scratch
                lostage = phase_d.enter_context(
                    tc.tile_pool(name="lost", bufs=2))
                for t in range(NT):
                    for hh in range(2):
                        h4 = 2 * t + hh
                        rows = slice(hh * 64, hh * 64 + 64)
                        qb = slice(h4 * 128, h4 * 128 + 64)
                        qb2 = slice(h4 * 128 + 64, h4 * 128 + 128)
                        k1b = slice((4 + h4) * 128, (4 + h4) * 128 + 64)
                        k1b2 = slice((4 + h4) * 128 + 64, (4 + h4) * 128 + 128)
                        k2b = slice((8 + h4) * 128, (8 + h4) * 128 + 64)
                        k2b2 = slice((8 + h4) * 128 + 64, (8 + h4) * 128 + 128)
                        hi = lostage.tile([128, S], BF16, tag="hi", name="hi")
                        lo = lostage.tile([128, S], BF16, tag="lo", name="lo")
                        nc.vector.tensor_copy(hi[rows, :], qhat[t][rows, :])
                        nc.vector.tensor_sub(lo[rows, :], qhat[t][rows, :],
                                             hi[rows, :])
                        nc.sync.dma_start(out=packs_dram[qb, :],
                                          in_=hi[rows, :])
                        nc.sync.dma_start(out=packs_dram[qb2, :],
                                          in_=lo[rows, :])
                        hi2 = lostage.tile([128, S], BF16, tag="hi",
                                           name="hi2")
                        lo2 = lostage.tile([128, S], BF16, tag="lo",
                                           name="lo2")
                        nc.vector.tensor_copy(hi2[rows, :], khat[t][rows, :])
                        nc.vector.tensor_sub(lo2[rows, :], khat[t][rows, :],
                                             hi2[rows, :])
                        nc.sync.dma_start(out=packs_dram[k1b, :],
                                          in_=hi2[rows, :])
                        nc.sync.dma_start(out=packs_dram[k1b2, :],
                                          in_=hi2[rows, :])
                        nc.sync.dma_start(out=packs_dram[k2b, :],
                                          in_=lo2[rows, :])
                        nc.sync.dma_start(out=packs_dram[k2b2, :],
                                          in_=lo2[rows, :])

                if DEBUG_DUMPS:
                    nc.sync.dma_start(out=dbg_q[:], in_=qhat[0])
                    nc.sync.dma_start(out=dbg_k[:], in_=khat[0])
                    nc.sync.dma_start(out=dbg_rq[:], in_=rq_all[0])

        if DEBUG_DUMPS:
            nc.sync.dma_start(out=dbg_v[:], in_=v_nat[0])
            nc.sync.dma_start(out=dbg_rc[:], in_=r_col)

        opool = ctx.enter_context(tc.tile_pool(name="o", bufs=1))
        outT = [opool.tile([128, S], _OF, tag=f"oT{k}", name=f"oT{k}")
                for k in range(2)]
        packp = ctx.enter_context(tc.tile_pool(name="packs", bufs=1))
        q_pack = [packp.tile([128, S], BF16, tag=f"qp{h}", name=f"qp{h}")
                  for h in range(HPC)]
        k1_pack = [packp.tile([128, S], BF16, tag=f"k1p{h}", name=f"k1p{h}")
                   for h in range(HPC)]
        k2_pack = [packp.tile([128, S], BF16, tag=f"k2p{h}", name=f"k2p{h}")
                   for h in range(HPC)]
        for h in range(HPC):
            nc.sync.dma_start(out=q_pack[h],
                              in_=packs_dram[h * 128:(h + 1) * 128, :])
            nc.sync.dma_start(out=k1_pack[h],
                              in_=packs_dram[(4 + h) * 128:(5 + h) * 128, :])
            nc.sync.dma_start(out=k2_pack[h],
                              in_=packs_dram[(8 + h) * 128:(9 + h) * 128, :])

        # ---- phase F: attention ------------------------------------------
        with ExitStack() as phase_f:
            dots_pool = phase_f.enter_context(
                tc.tile_pool(name="dots", bufs=5, space="PSUM"))
            tpps = phase_f.enter_context(
                tc.tile_pool(name="tpps", bufs=2, space="PSUM"))
            avps = phase_f.enter_context(
                tc.tile_pool(name="avps", bufs=1, space="PSUM"))
            attn_pool = phase_f.enter_context(tc.tile_pool(name="attn",
                                                           bufs=3))
            attnT_pool = phase_f.enter_context(tc.tile_pool(name="attnT",
                                                            bufs=2))
            small = phase_f.enter_context(tc.tile_pool(name="small", bufs=8))

            for t in range(NT):
                for hh in range(2):
                    h4 = 2 * t + hh
                    for sup in range(SCH // 4):
                        attnT = attnT_pool.tile([128, 4 * S], BF16,
                                                tag="attnT", name="attnT")
                        for u in range(4):
                            ic = sup * 4 + u
                            isl = slice(ic * 128, (ic + 1) * 128)
                            col = slice(2 * ic + hh, 2 * ic + hh + 1)
                            dots = [dots_pool.tile([128, 512], F32,
                                                   tag="dots", name="dots")
                                    for _ in range(NJ)]
                            for jn in range(NJ):
                                jsl = slice(jn * 512, (jn + 1) * 512)
                                nc.tensor.matmul(
                                    dots[jn], q_pack[h4][:, isl],
                                    k1_pack[h4][:, jsl],
                                    start=True, stop=False,
                                    skip_group_check=True)
                                nc.tensor.matmul(
                                    dots[jn], q_pack[h4][:, isl],
                                    k2_pack[h4][:, jsl],
                                    start=False, stop=True,
                                    skip_group_check=True)
                            mx = [small.tile([128, 1], F32, tag=f"mx{j}",
                                             name=f"mx{j}")
                                  for j in range(NJ)]
                            for jn in range(NJ):
                                nc.vector.tensor_reduce(out=mx[jn],
                                                        in_=dots[jn],
                                                        axis=AX.X, op=ALU.max)
                            nc.vector.tensor_max(mx[0], mx[0], mx[1])
                            nc.vector.tensor_max(mx[2], mx[2], mx[3])
                            nc.vector.tensor_max(mx[0], mx[0], mx[2])
                            bias = small.tile([128, 1], F32, tag="bias",
                                              name="bias")
                            nc.vector.tensor_mul(bias, mx[0],
                                                 nrq_all[t][:, col])
                            attn = attn_pool.tile([128, S], BF16, tag="attn",
                                                  name="attn")
                            sm = [small.tile([128, 1], F32, tag=f"sm{j}",
                                             name=f"sm{j}")
                                  for j in range(NJ)]
                            for jn in range(NJ):
                                jsl = slice(jn * 512, (jn + 1) * 512)
                                nc.scalar.activation(
                                    out=attn[:, jsl], in_=dots[jn],
                                    func=AF.Exp, bias=bias,
                                    scale=rq_all[t][:, col],
                                    accum_out=sm[jn])
                            nc.vector.tensor_add(sm[0], sm[0], sm[1])
                            nc.vector.tensor_add(sm[2], sm[2], sm[3])
                            nc.vector.tensor_add(sm[0], sm[0], sm[2])
                            # store 1/sum; normalization deferred to out^T
                            nc.vector.reciprocal(rs_all[t][:, col], sm[0])
                            if DEBUG_DUMPS and t == 0 and hh == 0 and \
                                    sup == 0 and u == 0:
                                nc.sync.dma_start(out=dbg_at[:], in_=attn)
                            for jq in range(SCH // 8):
                                tp = tpps.tile([128, 1024], BF16, tag="tp",
                                               name="tp")
                                for j2 in range(8):
                                    jc = jq * 8 + j2
                                    nc.tensor.transpose(
                                        tp[:, j2 * 128:(j2 + 1) * 128],
                                        attn[:, jc * 128:(jc + 1) * 128],
                                        ident)
                                # one strided copy per staging tile: the 8
                                # blocks land at jc*512 + u*128 in attnT
                                eng = nc.vector if jq % 2 == 0 else nc.scalar
                                src = tp[:].rearrange("p (b c) -> p b c", b=8)
                                dst = bass.AP(
                                    tensor=attnT.tensor,
                                    offset=attnT.offset
                                    + jq * 8 * 512 + u * 128,
                                    ap=[attnT.ap[0], [512, 8], [1, 128]],
                                )
                                if jq % 2 == 0:
                                    nc.vector.tensor_copy(dst, src)
                                else:
                                    nc.scalar.copy(dst, src)
                        av = avps.tile([64, 512], F32, tag="av", name="av")
                        for jc in range(SCH):
                            nc.tensor.matmul(
                                av, v_nat[jc][:, h4 * 64:(h4 + 1) * 64],
                                attnT[:, jc * 512:(jc + 1) * 512],
                                start=(jc == 0), stop=(jc == SCH - 1))
                        poff = hh * 64
                        ssl = slice(sup * 512, (sup + 1) * 512)
                        if sup % 2 == 0:
                            nc.vector.tensor_copy(outT[t][poff:poff + 64, ssl],
                                                  av)
                        else:
                            nc.scalar.copy(outT[t][poff:poff + 64, ssl], av)

        # ---- phase F2: normalize out^T by 1/sum ---------------------------
        with ExitStack() as phase_f2:
            rowps = phase_f2.enter_context(
                tc.tile_pool(name="rowps", bufs=2, space="PSUM"))
            bcps = phase_f2.enter_context(
                tc.tile_pool(name="bcps", bufs=2, space="PSUM"))
            rrow = phase_f2.enter_context(tc.tile_pool(name="rrow", bufs=2))
            for t in range(NT):
                rowA = rrow.tile([1, S], F32, tag="rowA", name="rowA")
                rowB = rrow.tile([1, S], F32, tag="rowB", name="rowB")
                for hh, row in ((0, rowA), (1, rowB)):
                    for nq in range(NJ):
                        rp = rowps.tile([1, 512], F32, tag="rp", name="rp")
                        for sc4 in range(4):
                            sc = nq * 4 + sc4
                            col = slice(2 * sc + hh, 2 * sc + hh + 1)
                            nc.tensor.transpose(
                                rp[:, sc4 * 128:(sc4 + 1) * 128],
                                rs_all[t][:, col], ident_f)
                        nc.vector.tensor_copy(
                            row[:, nq * 512:(nq + 1) * 512], rp)
                rs2 = rrow.tile([2, S], F32, tag="rs2", name="rs2")
                nc.sync.dma_start(out=rs2[0:1, :], in_=rowA)
                nc.sync.dma_start(out=rs2[1:2, :], in_=rowB)
                for nq in range(NJ):
                    nsl = slice(nq * 512, (nq + 1) * 512)
                    bc = bcps.tile([128, 512], F32, tag="bc", name="bc")
                    nc.tensor.matmul(bc, e2t2, rs2[:, nsl],
                                     start=True, stop=True)
                    nc.vector.tensor_mul(outT[t][:, nsl], outT[t][:, nsl], bc)

        if DEBUG_DUMPS:
            nc.sync.dma_start(out=dbg_ot[:], in_=outT[0])

        # ---- phase G: output projection (f32r) ---------------------------
        with ExitStack() as phase_g:
            wops = phase_g.enter_context(
                tc.tile_pool(name="wops", bufs=4, space="PSUM"))
            gpool = phase_g.enter_context(tc.tile_pool(name="g", bufs=1))
            ostage = phase_g.enter_context(tc.tile_pool(name="ost", bufs=2))

            wo = [gpool.tile([128, DIM], _OF, tag=f"wo{k}", name=f"wo{k}")
                  for k in range(2)]
            for k in range(2):
                nc.sync.dma_start(out=wo[k], in_=wo_d[k * 128:(k + 1) * 128, :])
            for sc in range(SCH):
                ssl = slice(sc * 128, (sc + 1) * 128)
                ost = ostage.tile([128, DIM], F32, tag="ost", name="ost")
                for nn in range(2):
                    nsl = slice(nn * 512, (nn + 1) * 512)
                    ps = wops.tile([128, 512], F32, tag="wops", name="wops")
                    for kk in range(2):
                        nc.tensor.matmul(ps, outT[kk][:, ssl], wo[kk][:, nsl],
                                         start=(kk == 0), stop=(kk == 1))
                    if nn % 2 == 0:
                        nc.vector.tensor_copy(ost[:, nsl], ps)
                    else:
                        nc.scalar.copy(ost[:, nsl], ps)
                nc.sync.dma_start(out=out_d[ssl, :], in_=ost)

    _fix_multiwaits(nc)
    return nc


_NC = None


def _get_nc():
    global _NC
    if _NC is None:
        _NC = _build_program()
    return _NC


def kernel(x, ln_w, ln_b, Wq, Wkv, q_gamma, k_gamma, Wo):
    x = np.asarray(x, np.float32)
    ln_w = np.asarray(ln_w, np.float32)
    ln_b = np.asarray(ln_b, np.float32)
    Wq = np.asarray(Wq, np.float32)
    Wkv = np.asarray(Wkv, np.float32)
    q_gamma = np.asarray(q_gamma, np.float32)
    k_gamma = np.asarray(k_gamma, np.float32)
    Wo = np.asarray(Wo, np.float32)
    Wk_full = Wkv[:, :H * DH]
    Wv_full = Wkv[:, H * DH:]

    bf = ml_dtypes.bfloat16
    e2_host = np.zeros((128, 2), np.float32)
    e2_host[0:64, 0] = 1.0
    e2_host[64:128, 1] = 1.0
    e2t_host = np.ascontiguousarray(e2_host.T)

    def aug_weights(Wsl):
        # [1152, INC]: [ln_w*W; -colsum/1024; ln_b@W; zeros]
        Wt = ln_w[:, None] * Wsl
        out = np.zeros((9 * 128, INC), np.float32)
        out[:DIM] = Wt
        out[DIM] = -Wt.sum(axis=0) / DIM
        out[DIM + 1] = ln_b @ Wsl
        return out

    in_maps = []
    for c in range(NCORES):
        b = c // (NCORES // B)
        g0 = (c % (NCORES // B)) * HPC
        hsl = slice(g0 * DH, (g0 + HPC) * DH)
        xt_host = np.ascontiguousarray(x[b].T)
        in_maps.append({
            "xT": xt_host,
            "Wq": aug_weights(Wq[:, hsl]),
            "Wk": aug_weights(Wk_full[:, hsl]),
            "Wv": aug_weights(Wv_full[:, hsl]).astype(bf),
            "Wo": np.ascontiguousarray(Wo[hsl, :]) if _OF != BF16
                  else np.ascontiguousarray(Wo[hsl, :]).astype(bf),
            "gq": (8.0 * q_gamma[g0:g0 + HPC]).reshape(INC, 1).astype(np.float32),
            "gk": (8.0 * k_gamma[g0:g0 + HPC]).reshape(INC, 1).astype(np.float32),
            "E2": e2_host,
            "E2T": e2t_host,
        })

    res = run_bass_kernel_spmd(_get_nc(), in_maps, list(range(NCORES))).results
    gpb = NCORES // B
    out = np.zeros((B, S, DIM), np.float32)
    for b in range(B):
        acc = np.zeros((S, DIM), np.float32)
        for c in range(b * gpb, (b + 1) * gpb):
            acc += res[c]["out"]
        out[b] = acc
    return out
